# revision 2
# baseline (speedup 1.0000x reference)
"""Deformable transformer decoder layer — TRN2 Bass/Tile kernel (dev version).

Per-core layout: one batch sample per NeuronCore (8 cores, B=8).
Feature-major [C, tokens] layouts throughout.
"""
import numpy as np
import ml_dtypes
import concourse.bass as bass
import concourse.mybir as mybir
from concourse import bacc
from concourse.tile import TileContext
from concourse.masks import make_identity

F32 = mybir.dt.float32
BF = mybir.dt.bfloat16
I16 = mybir.dt.int16
AF = mybir.ActivationFunctionType
OP = mybir.AluOpType

B, PQ, C, NH, L, P, FF = 8, 900, 256, 8, 4, 4, 2048
SHAPES = [(100, 152), (50, 76), (25, 38), (13, 19)]
PK = sum(h * w for h, w in SHAPES)
STARTS = [0, 15200, 19000, 19950]
DH = C // NH
QCH = 180          # query chunk for sampling phase
NQC = PQ // QCH    # 5 chunks
NCH = [512] * 39 + [PK - 39 * 512]  # key N-chunks (last = 229)

bf16 = ml_dtypes.bfloat16


def _perm_hlp():
    """device row (h,l,p) -> original channel (h*L+l)*P+p"""
    return np.array([(h * L + l) * P + p for h in range(NH) for l in range(L) for p in range(P)])


def _perm_off():
    """device rows: x-tile (h,l,p) then y-tile; orig ch ((h*L+l)*P+p)*2+xy"""
    hlp = _perm_hlp()
    return np.concatenate([hlp * 2, hlp * 2 + 1])


def _perm_pack():
    """even-set then odd-set: device row h*16+s holds chs (h*32+2s, h*32+2s+1)"""
    ev = np.array([h * 32 + 2 * s for h in range(NH) for s in range(16)])
    return ev, ev + 1


# ---- packed shared-constant layouts (one DRAM tensor each, shipped once
# per core instead of ~40 separate arrays: the axon tunnel charges ~90ms
# fixed cost per array) ----
CPACK_SPEC = [  # (name, rows, cols), f32
    ("b_in", 128, 6), ("b_out", 128, 2), ("bv", 128, 2), ("boff", 128, 2),
    ("baw", 128, 1), ("b_o", 128, 2), ("b1", 128, 16), ("b2", 128, 2),
    ("g1", 128, 2), ("be1", 128, 2), ("g2", 128, 2), ("be2", 128, 2),
    ("g3", 128, 2), ("be3", 128, 2),
    ("cW", 128, 1), ("cWm1", 128, 1), ("cHm1", 128, 1), ("cStart", 128, 1),
    ("ind_awsum", 128, 8), ("ind_awbc", 8, 128), ("ind_bc0", 8, 128),
    ("ind_bc1", 8, 128), ("onesrow", 1, 128), ("ind_refx", 8, 128),
    ("ind_refy", 8, 128),
]
CPACK_OFF = {}
_o = 0
for _n, _r, _c in CPACK_SPEC:
    CPACK_OFF[_n] = _o
    _o += _c
CPACK_COLS = _o

WPACK_SPEC = [  # (name, n_slabs_of_128_rows, cols), bf16
    ("winT", 2, 768), ("woutT", 2, 256), ("wvT", 2, 256), ("woffT", 2, 256),
    ("wawT", 2, 128), ("woTe", 1, 256), ("woTo", 1, 256),
    ("w1T", 2, 2048), ("w2T", 16, 256),
]
WPACK_OFF = {}
_o = 0
for _n, _s, _c in WPACK_SPEC:
    WPACK_OFF[_n] = _o
    _o += _s * _c
WPACK_COLS = _o


def host_prep(inputs: dict) -> list[dict]:
    """Full inputs -> list of 8 per-core input maps."""
    f = lambda a: np.ascontiguousarray(np.asarray(a), dtype=np.float32)
    b16 = lambda a: np.ascontiguousarray(np.asarray(a, dtype=np.float32).astype(bf16))
    col = lambda a: np.ascontiguousarray(f(a).reshape(-1, 128).T)  # [128, k], col m = tile m

    W_in, W_out, W_v, W_off, W_aw, W_o, W1, W2 = (np.asarray(inputs[k], np.float32)
        for k in ["W_in", "W_out", "W_v", "W_off", "W_aw", "W_o", "W1", "W2"])
    hlp = _perm_hlp()
    offp = _perm_off()
    ev, od = _perm_pack()

    shared = dict(
        winT=b16(W_in.T),                          # [256, 768]
        woutT=b16(W_out.T),                        # [256, 256]
        wvT=b16(W_v.T[:, np.concatenate([ev, od])]),  # [256, 256] cols: even-set|odd-set
        woffT=b16(W_off.T[:, offp]),               # [256, 256] cols: x(h,l,p)|y(h,l,p)
        wawT=b16(W_aw.T[:, hlp]),                  # [256, 128]
        woTe=b16(W_o.T[ev, :]),                    # [128, 256]
        woTo=b16(W_o.T[od, :]),                    # [128, 256]
        w1T=b16(W1.T),                             # [256, 2048]
        w2T=b16(W2.T),                             # [2048, 256]
        b_in=col(inputs["b_in"]), b_out=col(inputs["b_out"]),
        bv=col(np.asarray(inputs["b_v"], np.float32)[np.concatenate([ev, od])]),
        boff=col(np.asarray(inputs["b_off"], np.float32)[offp]),
        baw=col(np.asarray(inputs["b_aw"], np.float32)[hlp]),
        b_o=col(inputs["b_o"]), b1=col(inputs["b1"]), b2=col(inputs["b2"]),
        g1=col(inputs["g1"]), be1=col(inputs["be1"]), g2=col(inputs["g2"]),
        be2=col(inputs["be2"]), g3=col(inputs["g3"]), be3=col(inputs["be3"]),
    )
    # per-(h,l,p) constant columns
    Wl = np.array([SHAPES[l][1] for h in range(NH) for l in range(L) for p in range(P)], np.float32)
    Hl = np.array([SHAPES[l][0] for h in range(NH) for l in range(L) for p in range(P)], np.float32)
    St = np.array([STARTS[l] for h in range(NH) for l in range(L) for p in range(P)], np.float32)
    shared.update(cW=Wl.reshape(-1, 1), cWm1=(Wl - 1).reshape(-1, 1),
                  cHm1=(Hl - 1).reshape(-1, 1), cStart=St.reshape(-1, 1))
    # indicator lhsTs (f32)
    ind_awsum = np.zeros((128, 8), np.float32)
    for r in range(128):
        ind_awsum[r, r // 16] = 1.0
    ind_awbc = np.zeros((8, 128), np.float32)
    for m in range(128):
        ind_awbc[m // 16, m] = 1.0
    ind_bc0 = np.zeros((8, 128), np.float32)
    ind_bc1 = np.zeros((8, 128), np.float32)
    for m in range(128):
        ind_bc0[m // 32, m] = 1.0
        ind_bc1[4 + m // 32, m] = 1.0
    onesrow = np.ones((1, 128), np.float32)
    shared.update(ind_awsum=ind_awsum, ind_awbc=ind_awbc, ind_bc0=ind_bc0,
                  ind_bc1=ind_bc1, onesrow=onesrow)

    # per-(h,l,p) row -> level indicator for on-device refs expansion
    lidx = np.array([l for h in range(NH) for l in range(L) for p in range(P)])
    ind_refx = np.zeros((8, 128), np.float32)
    ind_refy = np.zeros((8, 128), np.float32)
    for r in range(128):
        ind_refx[lidx[r], r] = 1.0
        ind_refy[4 + lidx[r], r] = 1.0
    shared.update(ind_refx=ind_refx, ind_refy=ind_refy)
    # fold the "-0.5" of the coord computation into the offset bias
    shared["boff"] = shared["boff"] - 0.5

    # pack shared constants into two DRAM tensors
    cpack = np.zeros((128, CPACK_COLS), np.float32)
    for n, r, c in CPACK_SPEC:
        cpack[:r, CPACK_OFF[n]:CPACK_OFF[n] + c] = shared[n]
    wpack = np.zeros((128, WPACK_COLS), bf16)
    for n, s, c in WPACK_SPEC:
        w = shared[n]  # [128*s, c]
        o = WPACK_OFF[n]
        for k in range(s):
            wpack[:, o + k * c:o + (k + 1) * c] = w[128 * k:128 * (k + 1)]

    query_bf = np.asarray(inputs["query"], np.float32).astype(bf16)  # [B, 256, 900]
    key_bf = np.asarray(inputs["key"], np.float32).astype(bf16)      # [B, 256, PK]
    ref = np.asarray(inputs["reference_points"], np.float32)  # [B, 900, L, 2]

    # refs prescaled by level W/H so the device expansion is a plain matmul
    wlv = np.array([w for (h, w) in SHAPES], np.float32).reshape(L, 1)
    hlv = np.array([h for (h, w) in SHAPES], np.float32).reshape(L, 1)
    in_maps = []
    for b in range(B):
        refs = np.concatenate([ref[b, :, :, 0].T * wlv, ref[b, :, :, 1].T * hlv], 0)
        m = dict(xq=query_bf[b], keyt=key_bf[b],
                 refs=np.ascontiguousarray(refs, np.float32),
                 wpack=wpack, cpack=cpack)
        in_maps.append(m)
    return in_maps


DRAM_SPECS = dict(
    xq=([C, PQ], BF), keyt=([C, PK], BF),
    refs=([8, PQ], F32),
    wpack=([128, WPACK_COLS], BF),
    cpack=([128, CPACK_COLS], F32),
)

NSPLIT = [(0, 512), (512, 388)]  # (off, len) chunks of 900


def build(debug_outs=()):
    nc = bacc.Bacc("TRN2", target_bir_lowering=False, debug=False)
    D = {n: nc.dram_tensor(n, shp, dt, kind="ExternalInput") for n, (shp, dt) in DRAM_SPECS.items()}
    out_d = nc.dram_tensor("out", [C, PQ], BF, kind="ExternalOutput")
    dbg = {n: nc.dram_tensor("dbg_" + n, shp, F32, kind="ExternalOutput")
           for n, shp in debug_outs}

    with TileContext(nc) as tc:
        _emit(nc, tc, D, out_d, dbg)
    nc.compile()
    return nc


def _ln(nc, pools, x_tiles, g_ap, be_ap, out32, outbf):
    """LayerNorm over channel(partition) dim of 2x[128,900] f32 tiles.
    g_ap/be_ap: [256,1] sbuf tiles (sliced per 128). Writes f32 + bf16 outputs."""
    sb, ps, csts = pools
    onesf = csts["onesf"]       # [128,1] f32 ones
    onesrow = csts["onesrow"]   # [1,128] f32 ones
    psum_s = ps.tile([1, PQ], F32, tag="ln_s", name="ln_s")
    psum_q = ps.tile([1, PQ], F32, tag="ln_q", name="ln_q")
    xsq = sb.tile([128, PQ], F32, tag="ln_xsq", name="ln_xsq")
    for i in (0, 1):
        for (o, n) in NSPLIT:
            nc.tensor.matmul(psum_s[:, o:o + n], onesf[:], x_tiles[i][:, o:o + n],
                             start=(i == 0), stop=(i == 1))
    for i in (0, 1):
        nc.scalar.activation(out=xsq[:], in_=x_tiles[i][:], func=AF.Square)
        for (o, n) in NSPLIT:
            nc.tensor.matmul(psum_q[:, o:o + n], onesf[:], xsq[:, o:o + n],
                             start=(i == 0), stop=(i == 1))
    mean = sb.tile([1, PQ], F32, tag="ln_mean", name="ln_mean")
    nc.scalar.activation(out=mean[:], in_=psum_s[:], func=AF.Copy, scale=1.0 / 256)
    var = sb.tile([1, PQ], F32, tag="ln_var", name="ln_var")
    nc.vector.tensor_scalar(out=var[:], in0=psum_q[:], scalar1=1.0 / 256, scalar2=None, op0=OP.mult)
    m2 = sb.tile([1, PQ], F32, tag="ln_m2", name="ln_m2")
    nc.vector.tensor_tensor(out=m2[:], in0=mean[:], in1=mean[:], op=OP.mult)
    nc.vector.tensor_tensor(out=var[:], in0=var[:], in1=m2[:], op=OP.subtract)
    rv = sb.tile([1, PQ], F32, tag="ln_rv", name="ln_rv")
    nc.vector.tensor_scalar(out=var[:], in0=var[:], scalar1=1e-5, scalar2=None, op0=OP.add)
    nc.vector.reciprocal(rv[:], var[:])
    rstd = sb.tile([1, PQ], F32, tag="ln_rstd", name="ln_rstd")
    nc.scalar.activation(out=rstd[:], in_=rv[:], func=AF.Sqrt)
    # broadcast mean & rstd to 128 partitions via K=1 matmuls
    psum_mb = ps.tile([128, PQ], F32, tag="ln_mb", name="ln_mb")
    psum_rb = ps.tile([128, PQ], F32, tag="ln_rb", name="ln_rb")
    for (o, n) in NSPLIT:
        nc.tensor.matmul(psum_mb[:, o:o + n], onesrow[:], mean[:, o:o + n], start=True, stop=True)
    for (o, n) in NSPLIT:
        nc.tensor.matmul(psum_rb[:, o:o + n], onesrow[:], rstd[:, o:o + n], start=True, stop=True)
    for i in (0, 1):
        t = sb.tile([128, PQ], F32, tag="ln_t", name="ln_t")
        nc.vector.tensor_tensor(out=t[:], in0=x_tiles[i][:], in1=psum_mb[:], op=OP.subtract)
        nc.vector.tensor_tensor(out=t[:], in0=t[:], in1=psum_rb[:], op=OP.mult)
        nc.scalar.activation(out=out32[i][:], in_=t[:], func=AF.Identity,
                             scale=g_ap[:, i:i + 1], bias=be_ap[:, i:i + 1])
        if outbf is not None:
            nc.vector.tensor_copy(out=outbf[i][:], in_=out32[i][:])


def _emit(nc, tc, D, out_d, dbg):
    from contextlib import ExitStack
    _es = ExitStack()
    csts_pool = _es.enter_context(tc.tile_pool(name="consts", bufs=1))
    wpool = _es.enter_context(tc.tile_pool(name="weights", bufs=1))
    bigpool = _es.enter_context(tc.tile_pool(name="big", bufs=1))

    # ---- constants ----
    csts = {}
    iden = csts_pool.tile([128, 128], BF)
    make_identity(nc, iden)
    onesf = csts_pool.tile([128, 1], F32)
    nc.gpsimd.memset(onesf[:], 1.0)
    onesb = csts_pool.tile([128, 1], BF)
    nc.gpsimd.memset(onesb[:], 1.0)
    ind8b = csts_pool.tile([8, 128], BF)  # bf16 head-broadcast indicator
    small = {}
    for n, r, c in CPACK_SPEC:
        t = csts_pool.tile([r, c], F32, tag="c_" + n, name="c_")
        o = CPACK_OFF[n]
        nc.sync.dma_start(out=t[:], in_=D["cpack"][0:r, o:o + c])
        small[n] = t
    csts["onesf"] = onesf
    csts["onesrow"] = small["onesrow"]
    nc.vector.tensor_copy(out=ind8b[:], in_=small["ind_awbc"][:])

    # ---- weights to SBUF (bf16) from packed tensor ----
    W = {}
    for n, s, cols in WPACK_SPEC:
        if n in ("w1T", "w2T"):
            continue  # loaded in the FFN section
        tiles = []
        o = WPACK_OFF[n]
        for k in range(s):
            t = wpool.tile([128, cols], BF, tag=f"w_{n}{k}", name=f"w_{n}{k}")
            nc.sync.dma_start(out=t[:], in_=D["wpack"][:, o + k * cols:o + (k + 1) * cols])
            tiles.append(t)
        W[n] = tiles

    from contextlib import ExitStack as _ES2
    _es2 = _ES2()

    # ======== self-attention ========
    x1 = [bigpool.tile([128, PQ], F32, tag=f"xr_{i}", name=f"xr_{i}") for i in (0, 1)]
    with tc.tile_pool(name="qkvp", bufs=1) as qp, \
         tc.tile_pool(name="attn_sb", bufs=2) as asb, \
         tc.tile_pool(name="attn_big", bufs=1) as abig, \
         tc.tile_pool(name="attn_ps", bufs=2, space="PSUM") as aps, \
         tc.tile_pool(name="attn_ps1", bufs=1, space="PSUM") as aps1:
        xq32 = []
        xqb = []
        for i in (0, 1):
            tb = abig.tile([128, PQ], BF, tag=f"xqb_{i}", name=f"xqb_{i}")
            nc.sync.dma_start(out=tb[:], in_=D["xq"][128 * i:128 * (i + 1), :])
            xqb.append(tb)
            t = abig.tile([128, PQ], F32, tag=f"xq32_{i}", name=f"xq32_{i}")
            nc.scalar.activation(out=t[:], in_=tb[:], func=AF.Copy)
            xq32.append(t)
        qkvb = []
        for m in range(6):
            pq = aps.tile([128, PQ], F32, tag="psA", name="qkv_ps")
            for (o, n) in NSPLIT:
                for k in (0, 1):
                    nc.tensor.matmul(pq[:, o:o + n], W["winT"][k][:, 128 * m:128 * (m + 1)],
                                     xqb[k][:, o:o + n], start=(k == 0), stop=(k == 1))
            t = qp.tile([128, PQ], BF, tag=f"qkv_{m}", name=f"qkv_{m}")
            nc.scalar.activation(out=t[:], in_=pq[:], func=AF.Identity,
                                 bias=small["b_in"][:, m:m + 1])
            qkvb.append(t)

        SCH = [(0, 128), (128, 128), (256, 128), (384, 128), (512, 128),
               (640, 128), (768, 128), (896, 4)]
        rsum = abig.tile([8, PQ], F32, tag="rsum", name="rsum")
        attn_raw = [abig.tile([128, PQ], BF, tag=f"attnraw{i}", name=f"attnraw{i}") for i in (0, 1)]
        for h in range(NH):
            ti, ro = h // 4, (h % 4) * 32
            q_h = asb.tile([32, PQ], BF, tag="q_h", name="q_h")
            k_h = asb.tile([32, PQ], BF, tag="k_h", name="k_h")
            v_h = asb.tile([32, 1024], BF, tag="v_h", name="v_h")
            nc.sync.dma_start(out=q_h[:], in_=qkvb[0 + ti][ro:ro + 32, :])
            nc.sync.dma_start(out=k_h[:], in_=qkvb[2 + ti][ro:ro + 32, :])
            nc.sync.dma_start(out=v_h[:, :PQ], in_=qkvb[4 + ti][ro:ro + 32, :])
            nc.gpsimd.memset(v_h[:, PQ:], 0.0)
            expS = []
            for s, (so, sn) in enumerate(SCH):
                pS = aps.tile([128, PQ], F32, tag="psA", name="ps_S")
                for (o, n) in NSPLIT:
                    nc.tensor.matmul(pS[:sn, o:o + n], k_h[:, so:so + sn], q_h[:, o:o + n],
                                     start=True, stop=True)
                eS = asb.tile([128, PQ], BF, tag=f"expS{s}", name=f"expS{s}", bufs=2)
                nc.scalar.activation(out=eS[:sn, :], in_=pS[:sn, :], func=AF.Exp,
                                     scale=float(1.0 / np.sqrt(DH)))
                expS.append(eS)
            # sum over keys: bf16 tree + ones matmuls
            b1_ = asb.tile([128, PQ], BF, tag="sum_b1", name="sum_b1")
            b2_ = asb.tile([128, PQ], BF, tag="sum_b2", name="sum_b2")
            b3_ = asb.tile([128, PQ], BF, tag="sum_b3", name="sum_b3")
            nc.vector.tensor_tensor(out=b1_[:], in0=expS[0][:], in1=expS[1][:], op=OP.add)
            nc.vector.tensor_tensor(out=b2_[:], in0=expS[2][:], in1=expS[3][:], op=OP.add)
            nc.vector.tensor_tensor(out=b3_[:], in0=expS[4][:], in1=expS[5][:], op=OP.add)
            nc.vector.tensor_tensor(out=b1_[:], in0=b1_[:], in1=b2_[:], op=OP.add)
            nc.vector.tensor_tensor(out=b1_[:], in0=b1_[:], in1=b3_[:], op=OP.add)
            pssum = aps1.tile([1, PQ], F32, tag="ps_sum", name="ps_sum")
            for (o, n) in NSPLIT:
                nc.tensor.matmul(pssum[:, o:o + n], onesb[:], b1_[:, o:o + n], start=True, stop=False)
                nc.tensor.matmul(pssum[:, o:o + n], onesb[:], expS[6][:, o:o + n], start=False, stop=False)
                nc.tensor.matmul(pssum[:, o:o + n], onesb[0:4, :], expS[7][0:4, o:o + n], start=False, stop=True)
            sums1 = asb.tile([1, PQ], F32, tag="sums1", name="sums1")
            nc.scalar.activation(out=sums1[:], in_=pssum[:], func=AF.Copy)
            nc.sync.dma_start(out=rsum[h:h + 1, :], in_=sums1[:])
            # vT via DMA transpose
            vT = abig.tile([128, 8, 32], BF, tag="vT", name="vT")
            for s, (so, sn) in enumerate(SCH):
                nc.sync.dma_start(out=vT[:128, s, :], in_=v_h[:, so:so + 128], transpose=True)
            # attn @ v (unnormalized)
            pO = aps1.tile([32, PQ], F32, tag="ps_O", name="ps_O")
            for (o, n) in NSPLIT:
                for s, (so, sn) in enumerate(SCH):
                    nc.tensor.matmul(pO[:, o:o + n], vT[:sn, s, :], expS[s][:sn, o:o + n],
                                     start=(s == 0), stop=(s == 7))
            ao_h = asb.tile([32, PQ], BF, tag="ao_h", name="ao_h")
            nc.scalar.activation(out=ao_h[:], in_=pO[:], func=AF.Copy)
            nc.sync.dma_start(out=attn_raw[ti][ro:ro + 32, :], in_=ao_h[:])
        # normalize by 1/rowsum, then W_out + residual
        rr = abig.tile([8, PQ], F32, tag="rr", name="rr")
        nc.vector.reciprocal(rr[:], rsum[:])
        attn_n = []
        for i in (0, 1):
            pB = aps.tile([128, PQ], F32, tag="psA", name="ps_bc")
            ind = small["ind_bc0"] if i == 0 else small["ind_bc1"]
            for (o, n) in NSPLIT:
                nc.tensor.matmul(pB[:, o:o + n], ind[:], rr[:, o:o + n], start=True, stop=True)
            t = abig.tile([128, PQ], BF, tag=f"attn_n{i}", name=f"attn_n{i}")
            nc.vector.tensor_tensor(out=t[:], in0=attn_raw[i][:], in1=pB[:], op=OP.mult)
            attn_n.append(t)
        for m in (0, 1):
            pW = aps.tile([128, PQ], F32, tag="psA", name="ps_wout")
            for (o, n) in NSPLIT:
                for k in (0, 1):
                    nc.tensor.matmul(pW[:, o:o + n], W["woutT"][k][:, 128 * m:128 * (m + 1)],
                                     attn_n[k][:, o:o + n], start=(k == 0), stop=(k == 1))
            t = asb.tile([128, PQ], F32, tag="wout_t", name="wout_t")
            nc.scalar.activation(out=t[:], in_=pW[:], func=AF.Identity,
                                 bias=small["b_out"][:, m:m + 1])
            nc.vector.tensor_tensor(out=x1[m][:], in0=xq32[m][:], in1=t[:], op=OP.add)

    # ======== LN1 ========
    q1f = [bigpool.tile([128, PQ], F32, tag=f"qf{i}", name=f"q1f{i}") for i in (0, 1)]
    q1b = [bigpool.tile([128, PQ], BF, tag=f"qb{i}", name=f"q1b{i}") for i in (0, 1)]
    with tc.tile_pool(name="ln1", bufs=2) as lsb, \
         tc.tile_pool(name="ln1p", bufs=1, space="PSUM") as lps:
        _ln(nc, (lsb, lps, csts), x1, small["g1"], small["be1"], q1f, q1b)

    if "q1" in dbg:
        for i in (0, 1):
            nc.sync.dma_start(out=dbg["q1"][128 * i:128 * (i + 1), :], in_=q1f[i][:])

    # ======== offsets / attention weights / sampling prep ========
    wdup = []   # per-corner [128, 1800] bf16 (q, c2)-dup
    idx16 = []  # per-corner [128, 900] int16
    with tc.tile_pool(name="samp_sb", bufs=1) as sb, \
         tc.tile_pool(name="samp_ps", bufs=2, space="PSUM") as sps:
        def proj(wname, m, bias, n_out_rows=128):
            ps = sps.tile([128, PQ], F32, tag="proj_ps", name="proj_ps")
            for (o, n) in NSPLIT:
                for k in (0, 1):
                    nc.tensor.matmul(ps[:n_out_rows, o:o + n],
                                     W[wname][k][:, 128 * m:128 * m + n_out_rows],
                                     q1b[k][:, o:o + n], start=(k == 0), stop=(k == 1))
            return ps

        # aw softmax
        psA = proj("wawT", 0, None)
        expA = sb.tile([128, PQ], F32, tag="expA", name="expA")
        nc.scalar.activation(out=expA[:], in_=psA[:], func=AF.Exp, bias=small["baw"][:])
        psGS = sps.tile([8, PQ], F32, tag="ps_gs", name="ps_gs", bufs=1)
        for (o, n) in NSPLIT:
            nc.tensor.matmul(psGS[:, o:o + n], small["ind_awsum"][:], expA[:, o:o + n],
                             start=True, stop=True)
        rGS = sb.tile([8, PQ], F32, tag="rGS", name="rGS")
        nc.vector.reciprocal(rGS[:], psGS[:])
        psGB = sps.tile([128, PQ], F32, tag="ps_gb", name="ps_gb", bufs=1)
        for (o, n) in NSPLIT:
            nc.tensor.matmul(psGB[:, o:o + n], small["ind_awbc"][:], rGS[:, o:o + n],
                             start=True, stop=True)
        awn = sb.tile([128, PQ], F32, tag="awn", name="awn")
        nc.vector.tensor_tensor(out=awn[:], in0=expA[:], in1=psGB[:], op=OP.mult)

        # x / y coordinates: W_off projection and the prescaled-refs
        # expansion accumulate into one psum; bias carries b_off - 0.5.
        refs_t = sb.tile([8, PQ], F32, tag="refs_t", name="refs_t")
        nc.sync.dma_start(out=refs_t[:], in_=D["refs"][:])

        def coord(m, ind_name):
            ps = sps.tile([128, PQ], F32, tag="proj_ps", name="proj_ps")
            for (o, n) in NSPLIT:
                for k in (0, 1):
                    nc.tensor.matmul(ps[:, o:o + n],
                                     W["woffT"][k][:, 128 * m:128 * (m + 1)],
                                     q1b[k][:, o:o + n], start=(k == 0), stop=False)
                nc.tensor.matmul(ps[:, o:o + n], small[ind_name][:], refs_t[:, o:o + n],
                                 start=False, stop=True)
            xv = sb.tile([128, PQ], F32, tag=f"coord_{m}", name=f"coord_{m}")
            nc.scalar.activation(out=xv[:], in_=ps[:], func=AF.Identity,
                                 bias=small["boff"][:, m:m + 1])
            return xv

        x = coord(0, "ind_refx")
        y = coord(1, "ind_refy")

        def split_floor(v, cm1, pfx):
            rnd = sb.tile([128, PQ], F32, tag=f"{pfx}_rnd", name=f"{pfx}_rnd")
            nc.vector.tensor_scalar(out=rnd[:], in0=v[:], scalar1=8388608.0,
                                    scalar2=8388608.0, op0=OP.add, op1=OP.subtract)
            g_ = sb.tile([128, PQ], F32, tag=f"{pfx}_g", name=f"{pfx}_g")
            nc.vector.tensor_tensor(out=g_[:], in0=rnd[:], in1=v[:], op=OP.is_gt)
            i0 = sb.tile([128, PQ], F32, tag=f"{pfx}_i0", name=f"{pfx}_i0")
            nc.vector.tensor_tensor(out=i0[:], in0=rnd[:], in1=g_[:], op=OP.subtract)
            fr = sb.tile([128, PQ], F32, tag=f"{pfx}_fr", name=f"{pfx}_fr")
            nc.vector.tensor_tensor(out=fr[:], in0=v[:], in1=i0[:], op=OP.subtract)
            i0c = sb.tile([128, PQ], F32, tag=f"{pfx}_i0c", name=f"{pfx}_i0c")
            nc.vector.tensor_scalar(out=i0c[:], in0=i0[:], scalar1=0.0, scalar2=small[cm1][:],
                                    op0=OP.max, op1=OP.min)
            v0 = sb.tile([128, PQ], F32, tag=f"{pfx}_v0", name=f"{pfx}_v0")
            nc.vector.tensor_tensor(out=v0[:], in0=i0[:], in1=i0c[:], op=OP.is_equal)
            i1 = sb.tile([128, PQ], F32, tag=f"{pfx}_i1", name=f"{pfx}_i1")
            nc.vector.tensor_scalar(out=i1[:], in0=i0[:], scalar1=1.0, scalar2=None, op0=OP.add)
            i1c = sb.tile([128, PQ], F32, tag=f"{pfx}_i1c", name=f"{pfx}_i1c")
            nc.vector.tensor_scalar(out=i1c[:], in0=i1[:], scalar1=0.0, scalar2=small[cm1][:],
                                    op0=OP.max, op1=OP.min)
            v1 = sb.tile([128, PQ], F32, tag=f"{pfx}_v1", name=f"{pfx}_v1")
            nc.vector.tensor_tensor(out=v1[:], in0=i1[:], in1=i1c[:], op=OP.is_equal)
            return fr, i0c, v0, i1c, v1

        fx, x0c, vx0, x1c, vx1 = split_floor(x, "cWm1", "x")
        fy, y0c, vy0, y1c, vy1 = split_floor(y, "cHm1", "y")

        # weights
        wx1t = sb.tile([128, PQ], F32, tag="wx1t", name="wx1t")
        nc.vector.tensor_tensor(out=wx1t[:], in0=fx[:], in1=vx1[:], op=OP.mult)
        omfx = sb.tile([128, PQ], F32, tag="omfx", name="omfx")
        nc.scalar.activation(out=omfx[:], in_=fx[:], func=AF.Identity, scale=-1.0, bias=1.0)
        wx0t = sb.tile([128, PQ], F32, tag="wx0t", name="wx0t")
        nc.vector.tensor_tensor(out=wx0t[:], in0=omfx[:], in1=vx0[:], op=OP.mult)
        wy1t = sb.tile([128, PQ], F32, tag="wy1t", name="wy1t")
        nc.vector.tensor_tensor(out=wy1t[:], in0=fy[:], in1=vy1[:], op=OP.mult)
        nc.vector.tensor_tensor(out=wy1t[:], in0=wy1t[:], in1=awn[:], op=OP.mult)
        omfy = sb.tile([128, PQ], F32, tag="omfy", name="omfy")
        nc.scalar.activation(out=omfy[:], in_=fy[:], func=AF.Identity, scale=-1.0, bias=1.0)
        wy0t = sb.tile([128, PQ], F32, tag="wy0t", name="wy0t")
        nc.vector.tensor_tensor(out=wy0t[:], in0=omfy[:], in1=vy0[:], op=OP.mult)
        nc.vector.tensor_tensor(out=wy0t[:], in0=wy0t[:], in1=awn[:], op=OP.mult)

        corners = [(wx0t, wy0t), (wx1t, wy0t), (wx0t, wy1t), (wx1t, wy1t)]
        for ci, (wx, wy) in enumerate(corners):
            wd = bigpool.tile([128, PQ], BF, tag=f"wdup{ci}", name=f"wdup{ci}")
            nc.vector.tensor_tensor(out=wd[:], in0=wx[:], in1=wy[:], op=OP.mult)
            wdup.append(wd)

        # indices
        y0W = sb.tile([128, PQ], F32, tag="y0W", name="y0W")
        nc.vector.tensor_scalar(out=y0W[:], in0=y0c[:], scalar1=small["cW"][:],
                                scalar2=small["cStart"][:], op0=OP.mult, op1=OP.add)
        y1W = sb.tile([128, PQ], F32, tag="y1W", name="y1W")
        nc.vector.tensor_scalar(out=y1W[:], in0=y1c[:], scalar1=small["cW"][:],
                                scalar2=small["cStart"][:], op0=OP.mult, op1=OP.add)
        for ci, (yw, xc) in enumerate([(y0W, x0c), (y0W, x1c), (y1W, x0c), (y1W, x1c)]):
            idf = sb.tile([128, PQ], F32, tag="idf", name="idf")
            nc.vector.tensor_tensor(out=idf[:], in0=yw[:], in1=xc[:], op=OP.add)
            ii = bigpool.tile([128, PQ], I16, tag=f"idx{ci}", name=f"idx{ci}")
            nc.vector.tensor_copy(out=ii[:], in_=idf[:])
            idx16.append(ii)

        if "aw" in dbg:
            nc.sync.dma_start(out=dbg["aw"][:], in_=awn[:])
        if "xcoord" in dbg:
            nc.sync.dma_start(out=dbg["xcoord"][:], in_=x[:])

    # ======== value projection ========
    vpool = _es2.enter_context(tc.tile_pool(name="vpool", bufs=1))
    vtab = vpool.tile([128, PK], F32, tag="vtab", name="vtab")  # packed bf16-pairs as f32
    vtab_bf = vtab[:].bitcast(BF)                    # [128, 2*PK]
    with tc.tile_pool(name="vkey", bufs=3) as kp, \
         tc.tile_pool(name="vpsum", bufs=2, space="PSUM") as vps:
        off = 0
        for nlen in NCH:
            kb = kp.tile([128, 2, 512], BF, tag="keyb", name="keyb")
            for k in (0, 1):
                nc.gpsimd.dma_start(out=kb[:, k, :nlen], in_=D["keyt"][128 * k:128 * (k + 1), off:off + nlen])
            for m in (0, 1):  # even-set / odd-set
                pv = vps.tile([128, 512], F32, tag=f"vps{m}", name=f"vps{m}")
                for k in (0, 1):
                    nc.tensor.matmul(pv[:, :nlen], W["wvT"][k][:, 128 * m:128 * (m + 1)],
                                     kb[:, k, :nlen], start=(k == 0), stop=(k == 1))
                ov = vtab_bf[:, 2 * off + m: 2 * (off + nlen): 2]
                nc.scalar.activation(out=ov, in_=pv[:, :nlen], func=AF.Identity,
                                     bias=small["bv"][:, m:m + 1])
            off += nlen



    # ======== gather + combine ========
    samp = bigpool.tile([128, 2 * PQ], F32, tag="samp", name="samp")  # (q, c2) f32
    with tc.tile_pool(name="gat", bufs=1) as gp, \
         tc.tile_pool(name="gat2", bufs=1) as gp2, \
         tc.tile_pool(name="gat_ps", bufs=2, space="PSUM") as gps:
        HQ = QCH * 8  # cols per lp-half
        for qc in range(NQC):
            q0 = qc * QCH
            S_t = gp2.tile([128, QCH * 16 * 2], BF, tag="S_acc", name="S_acc")
            T_t = gp2.tile([128, QCH * 16 * 2], BF, tag="T_tmp", name="T_tmp")
            for ci in range(4):
                G = gp.tile([128, QCH * 16], F32, tag="G", name="G", bufs=2)
                nc.gpsimd.ap_gather(out_ap=G[:], in_ap=vtab[:], idxs_ap=idx16[ci][:, q0:q0 + QCH],
                                    channels=128, num_elems=PK, d=1, num_idxs=16 * QCH)
                # merge the 16 (l,p) weight rows of each head into one
                # partition (128 dma lines), then replicate across each
                # head's 16 channel-partitions with a K=8 PE matmul.
                M = gp.tile([8, QCH * 16], BF, tag="wmg", name="wmg", bufs=2)
                eng = nc.sync if ci % 2 == 0 else nc.scalar
                eng.dma_start(out=M[:], in_=wdup[ci][:, q0:q0 + QCH])
                Gv = G[:].bitcast(BF).rearrange("p (q lp c) -> p lp q c", q=QCH, lp=16, c=2)
                dst = S_t if ci == 0 else T_t
                dv = dst[:].rearrange("p (lp q c) -> p lp q c", q=QCH, lp=16, c=2)
                for s in (0, 1):
                    P_ = gps.tile([128, HQ], F32, tag="wrep", name="wrep")
                    for o in range(0, HQ, 512):
                        n = min(512, HQ - o)
                        nc.tensor.matmul(P_[:, o:o + n], ind8b[:],
                                         M[:, s * HQ + o:s * HQ + o + n],
                                         start=True, stop=True)
                    Pv = P_[:].rearrange("p (lp q) -> p lp q", lp=8)
                    for c_ in (0, 1):
                        nc.vector.tensor_tensor(out=dv[:, s * 8:(s + 1) * 8, :, c_],
                                                in0=Gv[:, s * 8:(s + 1) * 8, :, c_],
                                                in1=Pv, op=OP.mult)
                if ci > 0:
                    nc.vector.tensor_tensor(out=S_t[:], in0=S_t[:], in1=T_t[:], op=OP.add)
            # lp-tree: 16 -> 1
            sv = S_t[:].rearrange("p (lp x) -> p lp x", lp=16)
            t8 = gp2.tile([128, 8 * QCH * 2], BF, tag="t8", name="t8")
            t8v = t8[:].rearrange("p (lp x) -> p lp x", lp=8)
            nc.vector.tensor_tensor(out=t8v, in0=sv[:, 0:8], in1=sv[:, 8:16], op=OP.add)
            t4 = gp2.tile([128, 4 * QCH * 2], BF, tag="t4", name="t4")
            t4v = t4[:].rearrange("p (lp x) -> p lp x", lp=4)
            nc.vector.tensor_tensor(out=t4v, in0=t8v[:, 0:4], in1=t8v[:, 4:8], op=OP.add)
            t2 = gp2.tile([128, 2 * QCH * 2], BF, tag="t2", name="t2")
            t2v = t2[:].rearrange("p (lp x) -> p lp x", lp=2)
            nc.vector.tensor_tensor(out=t2v, in0=t4v[:, 0:2], in1=t4v[:, 2:4], op=OP.add)
            nc.vector.tensor_tensor(out=samp[:, 2 * q0:2 * (q0 + QCH)],
                                    in0=t2v[:, 0], in1=t2v[:, 1], op=OP.add)

    _es2.close()
    if "samp" in dbg:
        nc.sync.dma_start(out=dbg["samp"][:], in_=samp[:])

    # ======== W_o + residual + LN2 ========
    x2 = [bigpool.tile([128, PQ], F32, tag=f"xr_{i}", name=f"x2_{i}") for i in (0, 1)]
    with tc.tile_pool(name="wo_sb", bufs=2) as osb, \
         tc.tile_pool(name="wo_ps", bufs=2, space="PSUM") as ops_:
        sampb = osb.tile([128, 2 * PQ], BF, tag="sampb", name="sampb")
        nc.scalar.activation(out=sampb[:], in_=samp[:], func=AF.Copy)
        sv = sampb[:].rearrange("p (q c) -> p q c", c=2)
        for m in (0, 1):
            pW = ops_.tile([128, PQ], F32, tag="ps_wo", name="ps_wo")
            for (o, n) in NSPLIT:
                nc.tensor.matmul(pW[:, o:o + n], W["woTe"][0][:, 128 * m:128 * (m + 1)],
                                 sv[:, o:o + n, 0], start=True, stop=False)
                nc.tensor.matmul(pW[:, o:o + n], W["woTo"][0][:, 128 * m:128 * (m + 1)],
                                 sv[:, o:o + n, 1], start=False, stop=True)
            t = osb.tile([128, PQ], F32, tag="wo_t", name="wo_t")
            nc.scalar.activation(out=t[:], in_=pW[:], func=AF.Identity,
                                 bias=small["b_o"][:, m:m + 1])
            nc.vector.tensor_tensor(out=x2[m][:], in0=q1f[m][:], in1=t[:], op=OP.add)

    q2f = [bigpool.tile([128, PQ], F32, tag=f"qf{i}", name=f"q2f{i}") for i in (0, 1)]
    q2b = [bigpool.tile([128, PQ], BF, tag=f"qb{i}", name=f"q2b{i}") for i in (0, 1)]
    with tc.tile_pool(name="ln2", bufs=2) as lsb, \
         tc.tile_pool(name="ln2p", bufs=1, space="PSUM") as lps:
        _ln(nc, (lsb, lps, csts), x2, small["g2"], small["be2"], q2f, q2b)

    # ======== FFN + LN3 ========
    x3 = [bigpool.tile([128, PQ], F32, tag=f"xr_{i}", name=f"x3_{i}") for i in (0, 1)]
    with tc.tile_pool(name="ffw", bufs=1) as fw, \
         tc.tile_pool(name="ff_sb", bufs=1) as fsb, \
         tc.tile_pool(name="ff_ps", bufs=2, space="PSUM") as fps:
        w1t = []
        o1 = WPACK_OFF["w1T"]
        for k in (0, 1):
            t = fw.tile([128, FF], BF, tag=f"w1T{k}", name=f"w1T{k}")
            nc.sync.dma_start(out=t[:], in_=D["wpack"][:, o1 + k * FF:o1 + (k + 1) * FF])
            w1t.append(t)
        w2t = []
        o2 = WPACK_OFF["w2T"]
        for k in range(16):
            t = fw.tile([128, C], BF, tag=f"w2T{k}", name=f"w2T{k}")
            nc.sync.dma_start(out=t[:], in_=D["wpack"][:, o2 + k * C:o2 + (k + 1) * C])
            w2t.append(t)
        ffb = []
        for m in range(16):
            pF = fps.tile([128, PQ], F32, tag="ps_ff1", name="ps_ff1")
            for (o, n) in NSPLIT:
                for k in (0, 1):
                    nc.tensor.matmul(pF[:, o:o + n], w1t[k][:, 128 * m:128 * (m + 1)],
                                     q2b[k][:, o:o + n], start=(k == 0), stop=(k == 1))
            t = fsb.tile([128, PQ], BF, tag=f"ff_{m}", name=f"ff_{m}")
            nc.scalar.activation(out=t[:], in_=pF[:], func=AF.Relu,
                                 bias=small["b1"][:, m:m + 1])
            ffb.append(t)
        for m in (0, 1):
            pF2 = fps.tile([128, PQ], F32, tag="ps_ff2", name="ps_ff2")
            for (o, n) in NSPLIT:
                for k in range(16):
                    nc.tensor.matmul(pF2[:, o:o + n], w2t[k][:, 128 * m:128 * (m + 1)],
                                     ffb[k][:, o:o + n], start=(k == 0), stop=(k == 15))
            t = fsb.tile([128, PQ], F32, tag="ff2_t", name="ff2_t")
            nc.scalar.activation(out=t[:], in_=pF2[:], func=AF.Identity,
                                 bias=small["b2"][:, m:m + 1])
            nc.vector.tensor_tensor(out=x3[m][:], in0=q2f[m][:], in1=t[:], op=OP.add)

    outb = [bigpool.tile([128, PQ], BF, tag=f"qb{i}", name=f"o16_{i}") for i in (0, 1)]
    with tc.tile_pool(name="ln3", bufs=2) as lsb, \
         tc.tile_pool(name="ln3p", bufs=1, space="PSUM") as lps:
        _ln(nc, (lsb, lps, csts), x3, small["g3"], small["be3"], outb, None)
    for i in (0, 1):
        nc.sync.dma_start(out=out_d[128 * i:128 * (i + 1), :], in_=outb[i][:])

    _es.close()


# ======================================================================
# Self-contained entry point: kernel(**inputs) -> np.ndarray [B, C, PQ]
# Sharding: data-parallel over batch — one sample per NeuronCore (8 cores).
# ======================================================================
_CACHED = {}


def _get_nc():
    if "nc" not in _CACHED:
        _CACHED["nc"] = build()
    return _CACHED["nc"]


def kernel(**inputs) -> np.ndarray:
    from concourse.bass_utils import run_bass_kernel_spmd
    nc = _get_nc()
    in_maps = host_prep(inputs)
    res = run_bass_kernel_spmd(nc, in_maps, core_ids=list(range(B)))
    out = np.stack([res.results[b]["out"] for b in range(B)]).astype(np.float32)
    return out



# revision 4
# speedup vs baseline: 1.1498x; 1.1498x over previous
"""Deformable transformer decoder layer — TRN2 Bass/Tile kernel (dev version).

Per-core layout: one batch sample per NeuronCore (8 cores, B=8).
Feature-major [C, tokens] layouts throughout.
"""
import numpy as np
import ml_dtypes
import concourse.bass as bass
import concourse.mybir as mybir
from concourse import bacc
from concourse.tile import TileContext
from concourse.masks import make_identity

F32 = mybir.dt.float32
BF = mybir.dt.bfloat16
F8 = mybir.dt.float8e4
I16 = mybir.dt.int16
AF = mybir.ActivationFunctionType
OP = mybir.AluOpType

B, PQ, C, NH, L, P, FF = 8, 900, 256, 8, 4, 4, 2048
SHAPES = [(100, 152), (50, 76), (25, 38), (13, 19)]
PK = sum(h * w for h, w in SHAPES)
STARTS = [0, 15200, 19000, 19950]
DH = C // NH
QCH = 180          # query chunk for sampling phase
NQC = PQ // QCH    # 5 chunks
NCH = [512] * 39 + [PK - 39 * 512]  # key N-chunks (last = 229)

bf16 = ml_dtypes.bfloat16
f8e4 = ml_dtypes.float8_e4m3

_F8TAB = None


def _f8_table():
    """bf16 bits -> f8e4(8x value) lookup; 8x prescale keeps N(0,0.02) key
    data in e4m3's normal range (1/8 is folded into W_v)."""
    global _F8TAB
    if _F8TAB is None:
        with np.errstate(invalid="ignore", over="ignore"):
            vals = np.arange(65536, dtype=np.uint16).view(bf16).astype(np.float32) * 8.0
            _F8TAB = vals.astype(f8e4).view(np.uint8)
    return _F8TAB


def _perm_hlp():
    """device row (h,l,p) -> original channel (h*L+l)*P+p"""
    return np.array([(h * L + l) * P + p for h in range(NH) for l in range(L) for p in range(P)])


def _perm_off():
    """device rows: x-tile (h,l,p) then y-tile; orig ch ((h*L+l)*P+p)*2+xy"""
    hlp = _perm_hlp()
    return np.concatenate([hlp * 2, hlp * 2 + 1])


def _perm_pack():
    """even-set then odd-set: device row h*16+s holds chs (h*32+2s, h*32+2s+1)"""
    ev = np.array([h * 32 + 2 * s for h in range(NH) for s in range(16)])
    return ev, ev + 1


# ---- packed shared-constant layouts (one DRAM tensor each, shipped once
# per core instead of ~40 separate arrays: the axon tunnel charges ~90ms
# fixed cost per array) ----
CPACK_SPEC = [  # (name, rows, cols), f32
    ("b_in", 128, 6), ("b_out", 128, 2), ("bv", 128, 2), ("boff", 128, 2),
    ("baw", 128, 1), ("b_o", 128, 2), ("b1", 128, 16), ("b2", 128, 2),
    ("g1", 128, 2), ("be1", 128, 2), ("g2", 128, 2), ("be2", 128, 2),
    ("g3", 128, 2), ("be3", 128, 2),
    ("cW", 128, 1), ("cWm1", 128, 1), ("cHm1", 128, 1), ("cStart", 128, 1),
    ("ind_awsum", 128, 8), ("ind_awbc", 8, 128), ("ind_bc0", 8, 128),
    ("ind_bc1", 8, 128), ("onesrow", 1, 128), ("ind_refx", 8, 128),
    ("ind_refy", 8, 128),
]
CPACK_OFF = {}
_o = 0
for _n, _r, _c in CPACK_SPEC:
    CPACK_OFF[_n] = _o
    _o += _c
CPACK_COLS = _o

WPACK_SPEC = [  # (name, n_slabs_of_128_rows, cols), bf16
    ("winT", 2, 768), ("woutT", 2, 256), ("wvT", 2, 256), ("woffT", 2, 256),
    ("wawT", 2, 128), ("woTe", 1, 256), ("woTo", 1, 256),
    ("w1T", 2, 2048), ("w2T", 16, 256),
]
WPACK_OFF = {}
_o = 0
for _n, _s, _c in WPACK_SPEC:
    WPACK_OFF[_n] = _o
    _o += _s * _c
WPACK_COLS = _o


def host_prep(inputs: dict) -> list[dict]:
    """Full inputs -> list of 8 per-core input maps."""
    f = lambda a: np.ascontiguousarray(np.asarray(a), dtype=np.float32)
    b16 = lambda a: np.ascontiguousarray(np.asarray(a, dtype=np.float32).astype(bf16))
    col = lambda a: np.ascontiguousarray(f(a).reshape(-1, 128).T)  # [128, k], col m = tile m

    W_in, W_out, W_v, W_off, W_aw, W_o, W1, W2 = (np.asarray(inputs[k], np.float32)
        for k in ["W_in", "W_out", "W_v", "W_off", "W_aw", "W_o", "W1", "W2"])
    hlp = _perm_hlp()
    offp = _perm_off()
    ev, od = _perm_pack()

    shared = dict(
        winT=b16(W_in.T),                          # [256, 768]
        woutT=b16(W_out.T),                        # [256, 256]
        wvT=b16(W_v.T[:, np.concatenate([ev, od])] * 0.125),  # cols: even|odd; 1/8 of f8 key prescale
        woffT=b16(W_off.T[:, offp]),               # [256, 256] cols: x(h,l,p)|y(h,l,p)
        wawT=b16(W_aw.T[:, hlp]),                  # [256, 128]
        woTe=b16(W_o.T[ev, :]),                    # [128, 256]
        woTo=b16(W_o.T[od, :]),                    # [128, 256]
        w1T=b16(W1.T),                             # [256, 2048]
        w2T=b16(W2.T),                             # [2048, 256]
        b_in=col(inputs["b_in"]), b_out=col(inputs["b_out"]),
        bv=col(np.asarray(inputs["b_v"], np.float32)[np.concatenate([ev, od])]),
        boff=col(np.asarray(inputs["b_off"], np.float32)[offp]),
        baw=col(np.asarray(inputs["b_aw"], np.float32)[hlp]),
        b_o=col(inputs["b_o"]), b1=col(inputs["b1"]), b2=col(inputs["b2"]),
        g1=col(inputs["g1"]), be1=col(inputs["be1"]), g2=col(inputs["g2"]),
        be2=col(inputs["be2"]), g3=col(inputs["g3"]), be3=col(inputs["be3"]),
    )
    # per-(h,l,p) constant columns
    Wl = np.array([SHAPES[l][1] for h in range(NH) for l in range(L) for p in range(P)], np.float32)
    Hl = np.array([SHAPES[l][0] for h in range(NH) for l in range(L) for p in range(P)], np.float32)
    St = np.array([STARTS[l] for h in range(NH) for l in range(L) for p in range(P)], np.float32)
    shared.update(cW=Wl.reshape(-1, 1), cWm1=(Wl - 1).reshape(-1, 1),
                  cHm1=(Hl - 1).reshape(-1, 1), cStart=St.reshape(-1, 1))
    # indicator lhsTs (f32)
    ind_awsum = np.zeros((128, 8), np.float32)
    for r in range(128):
        ind_awsum[r, r // 16] = 1.0
    ind_awbc = np.zeros((8, 128), np.float32)
    for m in range(128):
        ind_awbc[m // 16, m] = 1.0
    ind_bc0 = np.zeros((8, 128), np.float32)
    ind_bc1 = np.zeros((8, 128), np.float32)
    for m in range(128):
        ind_bc0[m // 32, m] = 1.0
        ind_bc1[4 + m // 32, m] = 1.0
    onesrow = np.ones((1, 128), np.float32)
    shared.update(ind_awsum=ind_awsum, ind_awbc=ind_awbc, ind_bc0=ind_bc0,
                  ind_bc1=ind_bc1, onesrow=onesrow)

    # per-(h,l,p) row -> level indicator for on-device refs expansion
    lidx = np.array([l for h in range(NH) for l in range(L) for p in range(P)])
    ind_refx = np.zeros((8, 128), np.float32)
    ind_refy = np.zeros((8, 128), np.float32)
    for r in range(128):
        ind_refx[lidx[r], r] = 1.0
        ind_refy[4 + lidx[r], r] = 1.0
    shared.update(ind_refx=ind_refx, ind_refy=ind_refy)
    # fold the "-0.5" of the coord computation into the offset bias
    shared["boff"] = shared["boff"] - 0.5

    # pack shared constants into two DRAM tensors
    cpack = np.zeros((128, CPACK_COLS), np.float32)
    for n, r, c in CPACK_SPEC:
        cpack[:r, CPACK_OFF[n]:CPACK_OFF[n] + c] = shared[n]
    wpack = np.zeros((128, WPACK_COLS), bf16)
    for n, s, c in WPACK_SPEC:
        w = shared[n]  # [128*s, c]
        o = WPACK_OFF[n]
        for k in range(s):
            wpack[:, o + k * c:o + (k + 1) * c] = w[128 * k:128 * (k + 1)]

    query_bf = np.asarray(inputs["query"], np.float32).astype(bf16)  # [B, 256, 900]
    key_bf = np.asarray(inputs["key"], np.float32).astype(bf16)      # [B, 256, PK]
    key_f8 = _f8_table()[key_bf.view(np.uint16)].view(f8e4)          # 8x-prescaled
    ref = np.asarray(inputs["reference_points"], np.float32)  # [B, 900, L, 2]

    # refs prescaled by level W/H so the device expansion is a plain matmul
    wlv = np.array([w for (h, w) in SHAPES], np.float32).reshape(L, 1)
    hlv = np.array([h for (h, w) in SHAPES], np.float32).reshape(L, 1)
    in_maps = []
    for b in range(B):
        refs = np.concatenate([ref[b, :, :, 0].T * wlv, ref[b, :, :, 1].T * hlv], 0)
        m = dict(xq=query_bf[b], keyt=key_f8[b],
                 refs=np.ascontiguousarray(refs, np.float32),
                 wpack=wpack, cpack=cpack)
        in_maps.append(m)
    return in_maps


DRAM_SPECS = dict(
    xq=([C, PQ], BF), keyt=([C, PK], F8),
    refs=([8, PQ], F32),
    wpack=([128, WPACK_COLS], BF),
    cpack=([128, CPACK_COLS], F32),
)

NSPLIT = [(0, 512), (512, 388)]  # (off, len) chunks of 900


def build(debug_outs=()):
    nc = bacc.Bacc("TRN2", target_bir_lowering=False, debug=False)
    D = {n: nc.dram_tensor(n, shp, dt, kind="ExternalInput") for n, (shp, dt) in DRAM_SPECS.items()}
    out_d = nc.dram_tensor("out", [C, PQ], BF, kind="ExternalOutput")
    dbg = {n: nc.dram_tensor("dbg_" + n, shp, F32, kind="ExternalOutput")
           for n, shp in debug_outs}

    with TileContext(nc) as tc:
        _emit(nc, tc, D, out_d, dbg)
    nc.compile()
    return nc


def _ln(nc, pools, x_tiles, g_ap, be_ap, out32, outbf):
    """LayerNorm over channel(partition) dim of 2x[128,900] f32 tiles.
    g_ap/be_ap: [256,1] sbuf tiles (sliced per 128). Writes f32 + bf16 outputs."""
    sb, ps, csts = pools
    onesf = csts["onesf"]       # [128,1] f32 ones
    onesrow = csts["onesrow"]   # [1,128] f32 ones
    psum_s = ps.tile([1, PQ], F32, tag="ln_s", name="ln_s")
    psum_q = ps.tile([1, PQ], F32, tag="ln_q", name="ln_q")
    xsq = sb.tile([128, PQ], F32, tag="ln_xsq", name="ln_xsq")
    for i in (0, 1):
        for (o, n) in NSPLIT:
            nc.tensor.matmul(psum_s[:, o:o + n], onesf[:], x_tiles[i][:, o:o + n],
                             start=(i == 0), stop=(i == 1))
    for i in (0, 1):
        nc.scalar.activation(out=xsq[:], in_=x_tiles[i][:], func=AF.Square)
        for (o, n) in NSPLIT:
            nc.tensor.matmul(psum_q[:, o:o + n], onesf[:], xsq[:, o:o + n],
                             start=(i == 0), stop=(i == 1))
    mean = sb.tile([1, PQ], F32, tag="ln_mean", name="ln_mean")
    nc.scalar.activation(out=mean[:], in_=psum_s[:], func=AF.Copy, scale=1.0 / 256)
    var = sb.tile([1, PQ], F32, tag="ln_var", name="ln_var")
    nc.vector.tensor_scalar(out=var[:], in0=psum_q[:], scalar1=1.0 / 256, scalar2=None, op0=OP.mult)
    m2 = sb.tile([1, PQ], F32, tag="ln_m2", name="ln_m2")
    nc.vector.tensor_tensor(out=m2[:], in0=mean[:], in1=mean[:], op=OP.mult)
    nc.vector.tensor_tensor(out=var[:], in0=var[:], in1=m2[:], op=OP.subtract)
    rv = sb.tile([1, PQ], F32, tag="ln_rv", name="ln_rv")
    nc.vector.tensor_scalar(out=var[:], in0=var[:], scalar1=1e-5, scalar2=None, op0=OP.add)
    nc.vector.reciprocal(rv[:], var[:])
    rstd = sb.tile([1, PQ], F32, tag="ln_rstd", name="ln_rstd")
    nc.scalar.activation(out=rstd[:], in_=rv[:], func=AF.Sqrt)
    # broadcast mean & rstd to 128 partitions via K=1 matmuls
    psum_mb = ps.tile([128, PQ], F32, tag="ln_mb", name="ln_mb")
    psum_rb = ps.tile([128, PQ], F32, tag="ln_rb", name="ln_rb")
    for (o, n) in NSPLIT:
        nc.tensor.matmul(psum_mb[:, o:o + n], onesrow[:], mean[:, o:o + n], start=True, stop=True)
    for (o, n) in NSPLIT:
        nc.tensor.matmul(psum_rb[:, o:o + n], onesrow[:], rstd[:, o:o + n], start=True, stop=True)
    for i in (0, 1):
        t = sb.tile([128, PQ], F32, tag="ln_t", name="ln_t")
        nc.vector.tensor_tensor(out=t[:], in0=x_tiles[i][:], in1=psum_mb[:], op=OP.subtract)
        nc.vector.tensor_tensor(out=t[:], in0=t[:], in1=psum_rb[:], op=OP.mult)
        nc.scalar.activation(out=out32[i][:], in_=t[:], func=AF.Identity,
                             scale=g_ap[:, i:i + 1], bias=be_ap[:, i:i + 1])
        if outbf is not None:
            nc.vector.tensor_copy(out=outbf[i][:], in_=out32[i][:])


def _emit(nc, tc, D, out_d, dbg):
    from contextlib import ExitStack
    _es = ExitStack()
    csts_pool = _es.enter_context(tc.tile_pool(name="consts", bufs=1))
    wpool = _es.enter_context(tc.tile_pool(name="weights", bufs=1))
    bigpool = _es.enter_context(tc.tile_pool(name="big", bufs=1))

    # ---- constants ----
    csts = {}
    iden = csts_pool.tile([128, 128], BF)
    make_identity(nc, iden)
    onesf = csts_pool.tile([128, 1], F32)
    nc.gpsimd.memset(onesf[:], 1.0)
    onesb = csts_pool.tile([128, 1], BF)
    nc.gpsimd.memset(onesb[:], 1.0)
    ind8b = csts_pool.tile([8, 128], BF)  # bf16 head-broadcast indicator
    small = {}
    for n, r, c in CPACK_SPEC:
        t = csts_pool.tile([r, c], F32, tag="c_" + n, name="c_")
        o = CPACK_OFF[n]
        nc.sync.dma_start(out=t[:], in_=D["cpack"][0:r, o:o + c])
        small[n] = t
    csts["onesf"] = onesf
    csts["onesrow"] = small["onesrow"]
    nc.vector.tensor_copy(out=ind8b[:], in_=small["ind_awbc"][:])

    # ---- weights to SBUF (bf16) from packed tensor ----
    W = {}
    for n, s, cols in WPACK_SPEC:
        if n in ("w1T", "w2T"):
            continue  # loaded in the FFN section
        tiles = []
        o = WPACK_OFF[n]
        for k in range(s):
            t = wpool.tile([128, cols], BF, tag=f"w_{n}{k}", name=f"w_{n}{k}")
            nc.sync.dma_start(out=t[:], in_=D["wpack"][:, o + k * cols:o + (k + 1) * cols])
            tiles.append(t)
        W[n] = tiles

    from contextlib import ExitStack as _ES2
    _es2 = _ES2()

    # ======== value projection ========
    vpool = _es2.enter_context(tc.tile_pool(name="vpool", bufs=1))
    vtab = vpool.tile([128, PK], F32, tag="vtab", name="vtab")  # packed bf16-pairs as f32
    vtab_bf = vtab[:].bitcast(BF)                    # [128, 2*PK]
    with tc.tile_pool(name="vkey", bufs=3) as kp, \
         tc.tile_pool(name="vpsum", bufs=2, space="PSUM") as vps:
        off = 0
        for nlen in NCH:
            kb8 = kp.tile([128, 2, 512], F8, tag="keyb8", name="keyb8")
            for k in (0, 1):
                nc.gpsimd.dma_start(out=kb8[:, k, :nlen], in_=D["keyt"][128 * k:128 * (k + 1), off:off + nlen])
            kb = kp.tile([128, 2, 512], BF, tag="keyb", name="keyb")
            nc.scalar.activation(out=kb[:, :, :nlen], in_=kb8[:, :, :nlen], func=AF.Copy)
            for m in (0, 1):  # even-set / odd-set
                pv = vps.tile([128, 512], F32, tag=f"vps{m}", name=f"vps{m}")
                for k in (0, 1):
                    nc.tensor.matmul(pv[:, :nlen], W["wvT"][k][:, 128 * m:128 * (m + 1)],
                                     kb[:, k, :nlen], start=(k == 0), stop=(k == 1))
                ov = vtab_bf[:, 2 * off + m: 2 * (off + nlen): 2]
                nc.scalar.activation(out=ov, in_=pv[:, :nlen], func=AF.Identity,
                                     bias=small["bv"][:, m:m + 1])
            off += nlen


    # ======== self-attention ========
    x1 = [bigpool.tile([128, PQ], F32, tag=f"xr_{i}", name=f"xr_{i}") for i in (0, 1)]
    with tc.tile_pool(name="qkvp", bufs=1) as qp, \
         tc.tile_pool(name="attn_sb", bufs=2) as asb, \
         tc.tile_pool(name="attn_big", bufs=1) as abig, \
         tc.tile_pool(name="attn_ps", bufs=2, space="PSUM") as aps, \
         tc.tile_pool(name="attn_ps1", bufs=1, space="PSUM") as aps1:
        xq32 = []
        xqb = []
        for i in (0, 1):
            tb = abig.tile([128, PQ], BF, tag=f"xqb_{i}", name=f"xqb_{i}")
            nc.sync.dma_start(out=tb[:], in_=D["xq"][128 * i:128 * (i + 1), :])
            xqb.append(tb)
            t = abig.tile([128, PQ], F32, tag=f"xq32_{i}", name=f"xq32_{i}")
            nc.scalar.activation(out=t[:], in_=tb[:], func=AF.Copy)
            xq32.append(t)
        qkvb = []
        for m in range(6):
            pq = aps.tile([128, PQ], F32, tag="psA", name="qkv_ps")
            for (o, n) in NSPLIT:
                for k in (0, 1):
                    nc.tensor.matmul(pq[:, o:o + n], W["winT"][k][:, 128 * m:128 * (m + 1)],
                                     xqb[k][:, o:o + n], start=(k == 0), stop=(k == 1))
            t = qp.tile([128, PQ], BF, tag=f"qkv_{m}", name=f"qkv_{m}")
            nc.scalar.activation(out=t[:], in_=pq[:], func=AF.Identity,
                                 bias=small["b_in"][:, m:m + 1])
            qkvb.append(t)

        SCH = [(0, 128), (128, 128), (256, 128), (384, 128), (512, 128),
               (640, 128), (768, 128), (896, 4)]
        rsum = abig.tile([8, PQ], F32, tag="rsum", name="rsum")
        attn_raw = [abig.tile([128, PQ], BF, tag=f"attnraw{i}", name=f"attnraw{i}") for i in (0, 1)]
        for h in range(NH):
            ti, ro = h // 4, (h % 4) * 32
            q_h = asb.tile([32, PQ], BF, tag="q_h", name="q_h")
            k_h = asb.tile([32, PQ], BF, tag="k_h", name="k_h")
            v_h = asb.tile([32, 1024], BF, tag="v_h", name="v_h")
            nc.sync.dma_start(out=q_h[:], in_=qkvb[0 + ti][ro:ro + 32, :])
            nc.sync.dma_start(out=k_h[:], in_=qkvb[2 + ti][ro:ro + 32, :])
            nc.sync.dma_start(out=v_h[:, :PQ], in_=qkvb[4 + ti][ro:ro + 32, :])
            nc.gpsimd.memset(v_h[:, PQ:], 0.0)
            expS = []
            for s, (so, sn) in enumerate(SCH):
                pS = aps.tile([128, PQ], F32, tag="psA", name="ps_S")
                for (o, n) in NSPLIT:
                    nc.tensor.matmul(pS[:sn, o:o + n], k_h[:, so:so + sn], q_h[:, o:o + n],
                                     start=True, stop=True)
                eS = asb.tile([128, PQ], BF, tag=f"expS{s}", name=f"expS{s}", bufs=2)
                nc.scalar.activation(out=eS[:sn, :], in_=pS[:sn, :], func=AF.Exp,
                                     scale=float(1.0 / np.sqrt(DH)))
                expS.append(eS)
            # sum over keys: bf16 tree + ones matmuls
            b1_ = asb.tile([128, PQ], BF, tag="sum_b1", name="sum_b1")
            b2_ = asb.tile([128, PQ], BF, tag="sum_b2", name="sum_b2")
            b3_ = asb.tile([128, PQ], BF, tag="sum_b3", name="sum_b3")
            nc.vector.tensor_tensor(out=b1_[:], in0=expS[0][:], in1=expS[1][:], op=OP.add)
            nc.vector.tensor_tensor(out=b2_[:], in0=expS[2][:], in1=expS[3][:], op=OP.add)
            nc.vector.tensor_tensor(out=b3_[:], in0=expS[4][:], in1=expS[5][:], op=OP.add)
            nc.vector.tensor_tensor(out=b1_[:], in0=b1_[:], in1=b2_[:], op=OP.add)
            nc.vector.tensor_tensor(out=b1_[:], in0=b1_[:], in1=b3_[:], op=OP.add)
            pssum = aps1.tile([1, PQ], F32, tag="ps_sum", name="ps_sum")
            for (o, n) in NSPLIT:
                nc.tensor.matmul(pssum[:, o:o + n], onesb[:], b1_[:, o:o + n], start=True, stop=False)
                nc.tensor.matmul(pssum[:, o:o + n], onesb[:], expS[6][:, o:o + n], start=False, stop=False)
                nc.tensor.matmul(pssum[:, o:o + n], onesb[0:4, :], expS[7][0:4, o:o + n], start=False, stop=True)
            sums1 = asb.tile([1, PQ], F32, tag="sums1", name="sums1")
            nc.scalar.activation(out=sums1[:], in_=pssum[:], func=AF.Copy)
            nc.sync.dma_start(out=rsum[h:h + 1, :], in_=sums1[:])
            # vT via DMA transpose
            vT = abig.tile([128, 8, 32], BF, tag="vT", name="vT")
            for s, (so, sn) in enumerate(SCH):
                nc.sync.dma_start(out=vT[:128, s, :], in_=v_h[:, so:so + 128], transpose=True)
            # attn @ v (unnormalized)
            pO = aps1.tile([32, PQ], F32, tag="ps_O", name="ps_O")
            for (o, n) in NSPLIT:
                for s, (so, sn) in enumerate(SCH):
                    nc.tensor.matmul(pO[:, o:o + n], vT[:sn, s, :], expS[s][:sn, o:o + n],
                                     start=(s == 0), stop=(s == 7))
            ao_h = asb.tile([32, PQ], BF, tag="ao_h", name="ao_h")
            nc.scalar.activation(out=ao_h[:], in_=pO[:], func=AF.Copy)
            nc.sync.dma_start(out=attn_raw[ti][ro:ro + 32, :], in_=ao_h[:])
        # normalize by 1/rowsum, then W_out + residual
        nc.vector.reciprocal(rsum[:], rsum[:])
        rr = rsum
        attn_n = []
        for i in (0, 1):
            pB = aps.tile([128, PQ], F32, tag="psA", name="ps_bc")
            ind = small["ind_bc0"] if i == 0 else small["ind_bc1"]
            for (o, n) in NSPLIT:
                nc.tensor.matmul(pB[:, o:o + n], ind[:], rr[:, o:o + n], start=True, stop=True)
            t = abig.tile([128, PQ], BF, tag=f"attn_n{i}", name=f"attn_n{i}")
            nc.vector.tensor_tensor(out=t[:], in0=attn_raw[i][:], in1=pB[:], op=OP.mult)
            attn_n.append(t)
        for m in (0, 1):
            pW = aps.tile([128, PQ], F32, tag="psA", name="ps_wout")
            for (o, n) in NSPLIT:
                for k in (0, 1):
                    nc.tensor.matmul(pW[:, o:o + n], W["woutT"][k][:, 128 * m:128 * (m + 1)],
                                     attn_n[k][:, o:o + n], start=(k == 0), stop=(k == 1))
            t = asb.tile([128, PQ], F32, tag="wout_t", name="wout_t", bufs=1)
            nc.scalar.activation(out=t[:], in_=pW[:], func=AF.Identity,
                                 bias=small["b_out"][:, m:m + 1])
            nc.vector.tensor_tensor(out=x1[m][:], in0=xq32[m][:], in1=t[:], op=OP.add)

    # pool for tiles first written after attention (reuses attention SBUF)
    latepool = _es2.enter_context(tc.tile_pool(name="late", bufs=1))

    # ======== LN1 ========
    q1f = [bigpool.tile([128, PQ], F32, tag=f"qf{i}", name=f"q1f{i}") for i in (0, 1)]
    q1b = [bigpool.tile([128, PQ], BF, tag=f"qb{i}", name=f"q1b{i}") for i in (0, 1)]
    with tc.tile_pool(name="ln1", bufs=2) as lsb, \
         tc.tile_pool(name="ln1p", bufs=1, space="PSUM") as lps:
        _ln(nc, (lsb, lps, csts), x1, small["g1"], small["be1"], q1f, q1b)

    if "q1" in dbg:
        for i in (0, 1):
            nc.sync.dma_start(out=dbg["q1"][128 * i:128 * (i + 1), :], in_=q1f[i][:])

    # ======== offsets / attention weights / sampling prep ========
    wdup = []   # per-corner [128, 1800] bf16 (q, c2)-dup
    idx16 = []  # per-corner [128, 900] int16
    with tc.tile_pool(name="samp_sb", bufs=1) as sb, \
         tc.tile_pool(name="samp_ps", bufs=2, space="PSUM") as sps:
        def proj(wname, m, bias, n_out_rows=128):
            ps = sps.tile([128, PQ], F32, tag="proj_ps", name="proj_ps")
            for (o, n) in NSPLIT:
                for k in (0, 1):
                    nc.tensor.matmul(ps[:n_out_rows, o:o + n],
                                     W[wname][k][:, 128 * m:128 * m + n_out_rows],
                                     q1b[k][:, o:o + n], start=(k == 0), stop=(k == 1))
            return ps

        # x / y coordinates: W_off projection and the prescaled-refs
        # expansion accumulate into one psum; bias carries b_off - 0.5.
        refs_t = sb.tile([8, PQ], F32, tag="refs_t", name="refs_t")
        nc.sync.dma_start(out=refs_t[:], in_=D["refs"][:])

        def coord(m, ind_name):
            ps = sps.tile([128, PQ], F32, tag="proj_ps", name="proj_ps")
            for (o, n) in NSPLIT:
                for k in (0, 1):
                    nc.tensor.matmul(ps[:, o:o + n],
                                     W["woffT"][k][:, 128 * m:128 * (m + 1)],
                                     q1b[k][:, o:o + n], start=(k == 0), stop=False)
                nc.tensor.matmul(ps[:, o:o + n], small[ind_name][:], refs_t[:, o:o + n],
                                 start=False, stop=True)
            xv = sb.tile([128, PQ], F32, tag=f"coord_{m}", name=f"coord_{m}")
            nc.scalar.activation(out=xv[:], in_=ps[:], func=AF.Identity,
                                 bias=small["boff"][:, m:m + 1])
            return xv

        x = coord(0, "ind_refx")
        y = coord(1, "ind_refy")

        def split_floor(v, cm1, pfx):
            rnd = sb.tile([128, PQ], F32, tag="sf_rnd", name=f"{pfx}_rnd")
            nc.vector.tensor_scalar(out=rnd[:], in0=v[:], scalar1=8388608.0,
                                    scalar2=8388608.0, op0=OP.add, op1=OP.subtract)
            g_ = sb.tile([128, PQ], F32, tag="sf_g", name=f"{pfx}_g")
            nc.vector.tensor_tensor(out=g_[:], in0=rnd[:], in1=v[:], op=OP.is_gt)
            i0 = sb.tile([128, PQ], F32, tag="sf_i0", name=f"{pfx}_i0")
            nc.vector.tensor_tensor(out=i0[:], in0=rnd[:], in1=g_[:], op=OP.subtract)
            fr = sb.tile([128, PQ], F32, tag=f"{pfx}_fr", name=f"{pfx}_fr")
            nc.vector.tensor_tensor(out=fr[:], in0=v[:], in1=i0[:], op=OP.subtract)
            i0c = sb.tile([128, PQ], F32, tag=f"{pfx}_i0c", name=f"{pfx}_i0c")
            nc.vector.tensor_scalar(out=i0c[:], in0=i0[:], scalar1=0.0, scalar2=small[cm1][:],
                                    op0=OP.max, op1=OP.min)
            v0 = sb.tile([128, PQ], F32, tag=f"{pfx}_v0", name=f"{pfx}_v0")
            nc.vector.tensor_tensor(out=v0[:], in0=i0[:], in1=i0c[:], op=OP.is_equal)
            i1 = sb.tile([128, PQ], F32, tag="sf_i1", name=f"{pfx}_i1")
            nc.vector.tensor_scalar(out=i1[:], in0=i0[:], scalar1=1.0, scalar2=None, op0=OP.add)
            i1c = sb.tile([128, PQ], F32, tag=f"{pfx}_i1c", name=f"{pfx}_i1c")
            nc.vector.tensor_scalar(out=i1c[:], in0=i1[:], scalar1=0.0, scalar2=small[cm1][:],
                                    op0=OP.max, op1=OP.min)
            v1 = sb.tile([128, PQ], F32, tag=f"{pfx}_v1", name=f"{pfx}_v1")
            nc.vector.tensor_tensor(out=v1[:], in0=i1[:], in1=i1c[:], op=OP.is_equal)
            return fr, i0c, v0, i1c, v1

        fx, x0c, vx0, x1c, vx1 = split_floor(x, "cWm1", "x")
        fy, y0c, vy0, y1c, vy1 = split_floor(y, "cHm1", "y")

        # indices
        y0W = sb.tile([128, PQ], F32, tag="coord_0", name="y0W")
        nc.vector.tensor_scalar(out=y0W[:], in0=y0c[:], scalar1=small["cW"][:],
                                scalar2=small["cStart"][:], op0=OP.mult, op1=OP.add)
        y1W = sb.tile([128, PQ], F32, tag="coord_1", name="y1W")
        nc.vector.tensor_scalar(out=y1W[:], in0=y1c[:], scalar1=small["cW"][:],
                                scalar2=small["cStart"][:], op0=OP.mult, op1=OP.add)
        for ci, (yw, xc) in enumerate([(y0W, x0c), (y0W, x1c), (y1W, x0c), (y1W, x1c)]):
            idf = sb.tile([128, PQ], F32, tag="expA", name="idf")
            nc.vector.tensor_tensor(out=idf[:], in0=yw[:], in1=xc[:], op=OP.add)
            ii = latepool.tile([128, PQ], I16, tag=f"idx{ci}", name=f"idx{ci}")
            nc.vector.tensor_copy(out=ii[:], in_=idf[:])
            idx16.append(ii)

        # aw softmax
        psA = proj("wawT", 0, None)
        expA = sb.tile([128, PQ], F32, tag="expA", name="expA")
        nc.scalar.activation(out=expA[:], in_=psA[:], func=AF.Exp, bias=small["baw"][:])
        psGS = sps.tile([8, PQ], F32, tag="ps_gs", name="ps_gs", bufs=1)
        for (o, n) in NSPLIT:
            nc.tensor.matmul(psGS[:, o:o + n], small["ind_awsum"][:], expA[:, o:o + n],
                             start=True, stop=True)
        rGS = sb.tile([8, PQ], F32, tag="rGS", name="rGS")
        nc.vector.reciprocal(rGS[:], psGS[:])
        psGB = sps.tile([128, PQ], F32, tag="ps_gb", name="ps_gb", bufs=1)
        for (o, n) in NSPLIT:
            nc.tensor.matmul(psGB[:, o:o + n], small["ind_awbc"][:], rGS[:, o:o + n],
                             start=True, stop=True)
        awn = sb.tile([128, PQ], F32, tag="awn", name="awn")
        nc.vector.tensor_tensor(out=awn[:], in0=expA[:], in1=psGB[:], op=OP.mult)


        # weights, computed in place into the validity tiles
        wx1t = vx1
        nc.vector.tensor_tensor(out=wx1t[:], in0=fx[:], in1=vx1[:], op=OP.mult)
        omfx = sb.tile([128, PQ], F32, tag="omf", name="omfx")
        nc.scalar.activation(out=omfx[:], in_=fx[:], func=AF.Identity, scale=-1.0, bias=1.0)
        wx0t = vx0
        nc.vector.tensor_tensor(out=wx0t[:], in0=omfx[:], in1=vx0[:], op=OP.mult)
        wy1t = vy1
        nc.vector.tensor_tensor(out=wy1t[:], in0=fy[:], in1=vy1[:], op=OP.mult)
        nc.vector.tensor_tensor(out=wy1t[:], in0=wy1t[:], in1=awn[:], op=OP.mult)
        omfy = sb.tile([128, PQ], F32, tag="omf", name="omfy")
        nc.scalar.activation(out=omfy[:], in_=fy[:], func=AF.Identity, scale=-1.0, bias=1.0)
        wy0t = vy0
        nc.vector.tensor_tensor(out=wy0t[:], in0=omfy[:], in1=vy0[:], op=OP.mult)
        nc.vector.tensor_tensor(out=wy0t[:], in0=wy0t[:], in1=awn[:], op=OP.mult)

        corners = [(wx0t, wy0t), (wx1t, wy0t), (wx0t, wy1t), (wx1t, wy1t)]
        for ci, (wx, wy) in enumerate(corners):
            wd = latepool.tile([128, PQ], BF, tag=f"wdup{ci}", name=f"wdup{ci}")
            nc.vector.tensor_tensor(out=wd[:], in0=wx[:], in1=wy[:], op=OP.mult)
            wdup.append(wd)


        if "aw" in dbg:
            nc.sync.dma_start(out=dbg["aw"][:], in_=awn[:])
        if "xcoord" in dbg:
            nc.sync.dma_start(out=dbg["xcoord"][:], in_=x[:])


    # ======== gather + combine ========
    samp = latepool.tile([128, 2 * PQ], F32, tag="samp", name="samp")  # (q, c2) f32
    with tc.tile_pool(name="gat", bufs=1) as gp, \
         tc.tile_pool(name="gat2", bufs=1) as gp2, \
         tc.tile_pool(name="gat_ps", bufs=2, space="PSUM") as gps:
        HQ = QCH * 8  # cols per lp-half
        for qc in range(NQC):
            q0 = qc * QCH
            S_t = gp2.tile([128, QCH * 16 * 2], BF, tag="S_acc", name="S_acc")
            T_t = gp2.tile([128, QCH * 16 * 2], BF, tag="T_tmp", name="T_tmp")
            for ci in range(4):
                G = gp.tile([128, QCH * 16], F32, tag="G", name="G", bufs=2)
                nc.gpsimd.ap_gather(out_ap=G[:], in_ap=vtab[:], idxs_ap=idx16[ci][:, q0:q0 + QCH],
                                    channels=128, num_elems=PK, d=1, num_idxs=16 * QCH)
                # merge the 16 (l,p) weight rows of each head into one
                # partition (128 dma lines), then replicate across each
                # head's 16 channel-partitions with a K=8 PE matmul.
                M = gp.tile([8, QCH * 16], BF, tag="wmg", name="wmg", bufs=2)
                eng = nc.sync if ci % 2 == 0 else nc.scalar
                eng.dma_start(out=M[:], in_=wdup[ci][:, q0:q0 + QCH])
                Gv = G[:].bitcast(BF).rearrange("p (q lp c) -> p lp q c", q=QCH, lp=16, c=2)
                dst = S_t if ci == 0 else T_t
                dv = dst[:].rearrange("p (lp q c) -> p lp q c", q=QCH, lp=16, c=2)
                for s in (0, 1):
                    P_ = gps.tile([128, HQ], F32, tag="wrep", name="wrep")
                    for o in range(0, HQ, 512):
                        n = min(512, HQ - o)
                        nc.tensor.matmul(P_[:, o:o + n], ind8b[:],
                                         M[:, s * HQ + o:s * HQ + o + n],
                                         start=True, stop=True)
                    Pv = P_[:].rearrange("p (lp q) -> p lp q", lp=8)
                    for c_ in (0, 1):
                        nc.vector.tensor_tensor(out=dv[:, s * 8:(s + 1) * 8, :, c_],
                                                in0=Gv[:, s * 8:(s + 1) * 8, :, c_],
                                                in1=Pv, op=OP.mult)
                if ci > 0:
                    nc.vector.tensor_tensor(out=S_t[:], in0=S_t[:], in1=T_t[:], op=OP.add)
            # lp-tree: 16 -> 1
            sv = S_t[:].rearrange("p (lp x) -> p lp x", lp=16)
            t8 = gp2.tile([128, 8 * QCH * 2], BF, tag="t8", name="t8")
            t8v = t8[:].rearrange("p (lp x) -> p lp x", lp=8)
            nc.vector.tensor_tensor(out=t8v, in0=sv[:, 0:8], in1=sv[:, 8:16], op=OP.add)
            t4 = gp2.tile([128, 4 * QCH * 2], BF, tag="t4", name="t4")
            t4v = t4[:].rearrange("p (lp x) -> p lp x", lp=4)
            nc.vector.tensor_tensor(out=t4v, in0=t8v[:, 0:4], in1=t8v[:, 4:8], op=OP.add)
            t2 = gp2.tile([128, 2 * QCH * 2], BF, tag="t2", name="t2")
            t2v = t2[:].rearrange("p (lp x) -> p lp x", lp=2)
            nc.vector.tensor_tensor(out=t2v, in0=t4v[:, 0:2], in1=t4v[:, 2:4], op=OP.add)
            nc.vector.tensor_tensor(out=samp[:, 2 * q0:2 * (q0 + QCH)],
                                    in0=t2v[:, 0], in1=t2v[:, 1], op=OP.add)

    if "samp" in dbg:
        nc.sync.dma_start(out=dbg["samp"][:], in_=samp[:])

    # ======== W_o + residual + LN2 ========
    x2 = [bigpool.tile([128, PQ], F32, tag=f"xr_{i}", name=f"x2_{i}") for i in (0, 1)]
    with tc.tile_pool(name="wo_sb", bufs=2) as osb, \
         tc.tile_pool(name="wo_ps", bufs=2, space="PSUM") as ops_:
        sampb = osb.tile([128, 2 * PQ], BF, tag="sampb", name="sampb")
        nc.scalar.activation(out=sampb[:], in_=samp[:], func=AF.Copy)
        sv = sampb[:].rearrange("p (q c) -> p q c", c=2)
        for m in (0, 1):
            pW = ops_.tile([128, PQ], F32, tag="ps_wo", name="ps_wo")
            for (o, n) in NSPLIT:
                nc.tensor.matmul(pW[:, o:o + n], W["woTe"][0][:, 128 * m:128 * (m + 1)],
                                 sv[:, o:o + n, 0], start=True, stop=False)
                nc.tensor.matmul(pW[:, o:o + n], W["woTo"][0][:, 128 * m:128 * (m + 1)],
                                 sv[:, o:o + n, 1], start=False, stop=True)
            t = osb.tile([128, PQ], F32, tag="wo_t", name="wo_t")
            nc.scalar.activation(out=t[:], in_=pW[:], func=AF.Identity,
                                 bias=small["b_o"][:, m:m + 1])
            nc.vector.tensor_tensor(out=x2[m][:], in0=q1f[m][:], in1=t[:], op=OP.add)

    q2f = [bigpool.tile([128, PQ], F32, tag=f"qf{i}", name=f"q2f{i}") for i in (0, 1)]
    q2b = [bigpool.tile([128, PQ], BF, tag=f"qb{i}", name=f"q2b{i}") for i in (0, 1)]
    with tc.tile_pool(name="ln2", bufs=2) as lsb, \
         tc.tile_pool(name="ln2p", bufs=1, space="PSUM") as lps:
        _ln(nc, (lsb, lps, csts), x2, small["g2"], small["be2"], q2f, q2b)

    # ======== FFN + LN3 ========
    x3 = [bigpool.tile([128, PQ], F32, tag=f"xr_{i}", name=f"x3_{i}") for i in (0, 1)]
    with tc.tile_pool(name="ffw", bufs=1) as fw, \
         tc.tile_pool(name="ff_sb", bufs=1) as fsb, \
         tc.tile_pool(name="ff_ps", bufs=2, space="PSUM") as fps:
        w1t = []
        o1 = WPACK_OFF["w1T"]
        for k in (0, 1):
            t = fw.tile([128, FF], BF, tag=f"w1T{k}", name=f"w1T{k}")
            nc.sync.dma_start(out=t[:], in_=D["wpack"][:, o1 + k * FF:o1 + (k + 1) * FF])
            w1t.append(t)
        w2t = []
        o2 = WPACK_OFF["w2T"]
        for k in range(16):
            t = fw.tile([128, C], BF, tag=f"w2T{k}", name=f"w2T{k}")
            nc.sync.dma_start(out=t[:], in_=D["wpack"][:, o2 + k * C:o2 + (k + 1) * C])
            w2t.append(t)
        ffb = []
        for m in range(16):
            pF = fps.tile([128, PQ], F32, tag="ps_ff1", name="ps_ff1")
            for (o, n) in NSPLIT:
                for k in (0, 1):
                    nc.tensor.matmul(pF[:, o:o + n], w1t[k][:, 128 * m:128 * (m + 1)],
                                     q2b[k][:, o:o + n], start=(k == 0), stop=(k == 1))
            t = fsb.tile([128, PQ], BF, tag=f"ff_{m}", name=f"ff_{m}")
            nc.scalar.activation(out=t[:], in_=pF[:], func=AF.Relu,
                                 bias=small["b1"][:, m:m + 1])
            ffb.append(t)
        for m in (0, 1):
            pF2 = fps.tile([128, PQ], F32, tag="ps_ff2", name="ps_ff2")
            for (o, n) in NSPLIT:
                for k in range(16):
                    nc.tensor.matmul(pF2[:, o:o + n], w2t[k][:, 128 * m:128 * (m + 1)],
                                     ffb[k][:, o:o + n], start=(k == 0), stop=(k == 15))
            t = fsb.tile([128, PQ], F32, tag="ff2_t", name="ff2_t")
            nc.scalar.activation(out=t[:], in_=pF2[:], func=AF.Identity,
                                 bias=small["b2"][:, m:m + 1])
            nc.vector.tensor_tensor(out=x3[m][:], in0=q2f[m][:], in1=t[:], op=OP.add)

    outb = [bigpool.tile([128, PQ], BF, tag=f"qb{i}", name=f"o16_{i}") for i in (0, 1)]
    with tc.tile_pool(name="ln3", bufs=2) as lsb, \
         tc.tile_pool(name="ln3p", bufs=1, space="PSUM") as lps:
        _ln(nc, (lsb, lps, csts), x3, small["g3"], small["be3"], outb, None)
    for i in (0, 1):
        nc.sync.dma_start(out=out_d[128 * i:128 * (i + 1), :], in_=outb[i][:])

    _es2.close()
    _es.close()


# ======================================================================
# Self-contained entry point: kernel(**inputs) -> np.ndarray [B, C, PQ]
# Sharding: data-parallel over batch — one sample per NeuronCore (8 cores).
# ======================================================================
_CACHED = {}


def _get_nc():
    if "nc" not in _CACHED:
        _CACHED["nc"] = build()
    return _CACHED["nc"]


def kernel(**inputs) -> np.ndarray:
    from concourse.bass_utils import run_bass_kernel_spmd
    nc = _get_nc()
    in_maps = host_prep(inputs)
    res = run_bass_kernel_spmd(nc, in_maps, core_ids=list(range(B)))
    out = np.stack([res.results[b]["out"] for b in range(B)]).astype(np.float32)
    return out



# revision 5
# speedup vs baseline: 1.1677x; 1.0156x over previous
"""Deformable transformer decoder layer — TRN2 Bass/Tile kernel (dev version).

Per-core layout: one batch sample per NeuronCore (8 cores, B=8).
Feature-major [C, tokens] layouts throughout.
"""
import numpy as np
import ml_dtypes
import concourse.bass as bass
import concourse.mybir as mybir
from concourse import bacc
from concourse.tile import TileContext
from concourse.masks import make_identity

F32 = mybir.dt.float32
BF = mybir.dt.bfloat16
F8 = mybir.dt.float8e4
I16 = mybir.dt.int16
AF = mybir.ActivationFunctionType
OP = mybir.AluOpType

B, PQ, C, NH, L, P, FF = 8, 900, 256, 8, 4, 4, 2048
SHAPES = [(100, 152), (50, 76), (25, 38), (13, 19)]
PK = sum(h * w for h, w in SHAPES)
STARTS = [0, 15200, 19000, 19950]
DH = C // NH
QCH = 225          # query chunk for sampling phase
NQC = PQ // QCH    # 4 chunks
NCH = [512] * 39 + [PK - 39 * 512]  # key N-chunks (last = 229)

bf16 = ml_dtypes.bfloat16
f8e4 = ml_dtypes.float8_e4m3

_F8TAB = None


def _f8_table():
    """bf16 bits -> f8e4(8x value) lookup; 8x prescale keeps N(0,0.02) key
    data in e4m3's normal range (1/8 is folded into W_v)."""
    global _F8TAB
    if _F8TAB is None:
        with np.errstate(invalid="ignore", over="ignore"):
            vals = np.arange(65536, dtype=np.uint16).view(bf16).astype(np.float32) * 8.0
            _F8TAB = vals.astype(f8e4).view(np.uint8)
    return _F8TAB


def _perm_hlp():
    """device row (h,l,p) -> original channel (h*L+l)*P+p"""
    return np.array([(h * L + l) * P + p for h in range(NH) for l in range(L) for p in range(P)])


def _perm_off():
    """device rows: x-tile (h,l,p) then y-tile; orig ch ((h*L+l)*P+p)*2+xy"""
    hlp = _perm_hlp()
    return np.concatenate([hlp * 2, hlp * 2 + 1])


def _perm_pack():
    """even-set then odd-set: device row h*16+s holds chs (h*32+2s, h*32+2s+1)"""
    ev = np.array([h * 32 + 2 * s for h in range(NH) for s in range(16)])
    return ev, ev + 1


# ---- packed shared-constant layouts (one DRAM tensor each, shipped once
# per core instead of ~40 separate arrays: the axon tunnel charges ~90ms
# fixed cost per array) ----
CPACK_SPEC = [  # (name, rows, cols), f32
    ("b_in", 128, 6), ("b_out", 128, 2), ("bv", 128, 2), ("boff", 128, 2),
    ("baw", 128, 1), ("b_o", 128, 2), ("b1", 128, 16), ("b2", 128, 2),
    ("g1", 128, 2), ("be1", 128, 2), ("g2", 128, 2), ("be2", 128, 2),
    ("g3", 128, 2), ("be3", 128, 2),
    ("cW", 128, 1), ("cWm1", 128, 1), ("cHm1", 128, 1), ("cStart", 128, 1),
    ("ind_awsum", 128, 8), ("ind_awbc", 8, 128), ("ind_bc0", 8, 128),
    ("ind_bc1", 8, 128), ("onesrow", 1, 128), ("ind_refx", 8, 128),
    ("ind_refy", 8, 128),
]
CPACK_OFF = {}
_o = 0
for _n, _r, _c in CPACK_SPEC:
    CPACK_OFF[_n] = _o
    _o += _c
CPACK_COLS = _o

WPACK_SPEC = [  # (name, n_slabs_of_128_rows, cols), bf16
    ("winT", 2, 768), ("woutT", 2, 256), ("wvT", 2, 256), ("woffT", 2, 256),
    ("wawT", 2, 128), ("woTe", 1, 256), ("woTo", 1, 256),
    ("w1T", 2, 2048), ("w2T", 16, 256),
]
WPACK_OFF = {}
_o = 0
for _n, _s, _c in WPACK_SPEC:
    WPACK_OFF[_n] = _o
    _o += _s * _c
WPACK_COLS = _o


def host_prep(inputs: dict) -> list[dict]:
    """Full inputs -> list of 8 per-core input maps."""
    f = lambda a: np.ascontiguousarray(np.asarray(a), dtype=np.float32)
    b16 = lambda a: np.ascontiguousarray(np.asarray(a, dtype=np.float32).astype(bf16))
    col = lambda a: np.ascontiguousarray(f(a).reshape(-1, 128).T)  # [128, k], col m = tile m

    W_in, W_out, W_v, W_off, W_aw, W_o, W1, W2 = (np.asarray(inputs[k], np.float32)
        for k in ["W_in", "W_out", "W_v", "W_off", "W_aw", "W_o", "W1", "W2"])
    hlp = _perm_hlp()
    offp = _perm_off()
    ev, od = _perm_pack()

    shared = dict(
        winT=b16(W_in.T),                          # [256, 768]
        woutT=b16(W_out.T),                        # [256, 256]
        wvT=b16(W_v.T[:, np.concatenate([ev, od])] * 0.125),  # cols: even|odd; 1/8 of f8 key prescale
        woffT=b16(W_off.T[:, offp]),               # [256, 256] cols: x(h,l,p)|y(h,l,p)
        wawT=b16(W_aw.T[:, hlp]),                  # [256, 128]
        woTe=b16(W_o.T[ev, :]),                    # [128, 256]
        woTo=b16(W_o.T[od, :]),                    # [128, 256]
        w1T=b16(W1.T),                             # [256, 2048]
        w2T=b16(W2.T),                             # [2048, 256]
        b_in=col(inputs["b_in"]), b_out=col(inputs["b_out"]),
        bv=col(np.asarray(inputs["b_v"], np.float32)[np.concatenate([ev, od])]),
        boff=col(np.asarray(inputs["b_off"], np.float32)[offp]),
        baw=col(np.asarray(inputs["b_aw"], np.float32)[hlp]),
        b_o=col(inputs["b_o"]), b1=col(inputs["b1"]), b2=col(inputs["b2"]),
        g1=col(inputs["g1"]), be1=col(inputs["be1"]), g2=col(inputs["g2"]),
        be2=col(inputs["be2"]), g3=col(inputs["g3"]), be3=col(inputs["be3"]),
    )
    # per-(h,l,p) constant columns
    Wl = np.array([SHAPES[l][1] for h in range(NH) for l in range(L) for p in range(P)], np.float32)
    Hl = np.array([SHAPES[l][0] for h in range(NH) for l in range(L) for p in range(P)], np.float32)
    St = np.array([STARTS[l] for h in range(NH) for l in range(L) for p in range(P)], np.float32)
    shared.update(cW=Wl.reshape(-1, 1), cWm1=(Wl - 1).reshape(-1, 1),
                  cHm1=(Hl - 1).reshape(-1, 1), cStart=St.reshape(-1, 1))
    # indicator lhsTs (f32)
    ind_awsum = np.zeros((128, 8), np.float32)
    for r in range(128):
        ind_awsum[r, r // 16] = 1.0
    ind_awbc = np.zeros((8, 128), np.float32)
    for m in range(128):
        ind_awbc[m // 16, m] = 1.0
    ind_bc0 = np.zeros((8, 128), np.float32)
    ind_bc1 = np.zeros((8, 128), np.float32)
    for m in range(128):
        ind_bc0[m // 32, m] = 1.0
        ind_bc1[4 + m // 32, m] = 1.0
    onesrow = np.ones((1, 128), np.float32)
    shared.update(ind_awsum=ind_awsum, ind_awbc=ind_awbc, ind_bc0=ind_bc0,
                  ind_bc1=ind_bc1, onesrow=onesrow)

    # per-(h,l,p) row -> level indicator for on-device refs expansion
    lidx = np.array([l for h in range(NH) for l in range(L) for p in range(P)])
    ind_refx = np.zeros((8, 128), np.float32)
    ind_refy = np.zeros((8, 128), np.float32)
    for r in range(128):
        ind_refx[lidx[r], r] = 1.0
        ind_refy[4 + lidx[r], r] = 1.0
    shared.update(ind_refx=ind_refx, ind_refy=ind_refy)
    # fold the "-0.5" of the coord computation into the offset bias
    shared["boff"] = shared["boff"] - 0.5

    # pack shared constants into two DRAM tensors
    cpack = np.zeros((128, CPACK_COLS), np.float32)
    for n, r, c in CPACK_SPEC:
        cpack[:r, CPACK_OFF[n]:CPACK_OFF[n] + c] = shared[n]
    wpack = np.zeros((128, WPACK_COLS), bf16)
    for n, s, c in WPACK_SPEC:
        w = shared[n]  # [128*s, c]
        o = WPACK_OFF[n]
        for k in range(s):
            wpack[:, o + k * c:o + (k + 1) * c] = w[128 * k:128 * (k + 1)]

    query_bf = np.asarray(inputs["query"], np.float32).astype(bf16)  # [B, 256, 900]
    key_bf = np.asarray(inputs["key"], np.float32).astype(bf16)      # [B, 256, PK]
    key_f8 = _f8_table()[key_bf.view(np.uint16)].view(f8e4)          # 8x-prescaled
    ref = np.asarray(inputs["reference_points"], np.float32)  # [B, 900, L, 2]

    # refs prescaled by level W/H so the device expansion is a plain matmul
    wlv = np.array([w for (h, w) in SHAPES], np.float32).reshape(L, 1)
    hlv = np.array([h for (h, w) in SHAPES], np.float32).reshape(L, 1)
    in_maps = []
    for b in range(B):
        refs = np.concatenate([ref[b, :, :, 0].T * wlv, ref[b, :, :, 1].T * hlv], 0)
        m = dict(xq=query_bf[b], keyt=key_f8[b],
                 refs=np.ascontiguousarray(refs, np.float32),
                 wpack=wpack, cpack=cpack)
        in_maps.append(m)
    return in_maps


DRAM_SPECS = dict(
    xq=([C, PQ], BF), keyt=([C, PK], F8),
    refs=([8, PQ], F32),
    wpack=([128, WPACK_COLS], BF),
    cpack=([128, CPACK_COLS], F32),
)

NSPLIT = [(0, 512), (512, 388)]  # (off, len) chunks of 900


def build(debug_outs=()):
    nc = bacc.Bacc("TRN2", target_bir_lowering=False, debug=False)
    D = {n: nc.dram_tensor(n, shp, dt, kind="ExternalInput") for n, (shp, dt) in DRAM_SPECS.items()}
    out_d = nc.dram_tensor("out", [C, PQ], BF, kind="ExternalOutput")
    dbg = {n: nc.dram_tensor("dbg_" + n, shp, F32, kind="ExternalOutput")
           for n, shp in debug_outs}

    with TileContext(nc) as tc:
        _emit(nc, tc, D, out_d, dbg)
    nc.compile()
    return nc


def _ln(nc, pools, x_tiles, g_ap, be_ap, out32, outbf):
    """LayerNorm over channel(partition) dim of 2x[128,900] f32 tiles.
    g_ap/be_ap: [256,1] sbuf tiles (sliced per 128). Writes f32 + bf16 outputs."""
    sb, ps, csts = pools
    onesf = csts["onesf"]       # [128,1] f32 ones
    onesrow = csts["onesrow"]   # [1,128] f32 ones
    psum_s = ps.tile([1, PQ], F32, tag="ln_s", name="ln_s")
    psum_q = ps.tile([1, PQ], F32, tag="ln_q", name="ln_q")
    xsq = sb.tile([128, PQ], F32, tag="ln_xsq", name="ln_xsq")
    for i in (0, 1):
        for (o, n) in NSPLIT:
            nc.tensor.matmul(psum_s[:, o:o + n], onesf[:], x_tiles[i][:, o:o + n],
                             start=(i == 0), stop=(i == 1))
    for i in (0, 1):
        nc.scalar.activation(out=xsq[:], in_=x_tiles[i][:], func=AF.Square)
        for (o, n) in NSPLIT:
            nc.tensor.matmul(psum_q[:, o:o + n], onesf[:], xsq[:, o:o + n],
                             start=(i == 0), stop=(i == 1))
    mean = sb.tile([1, PQ], F32, tag="ln_mean", name="ln_mean")
    nc.scalar.activation(out=mean[:], in_=psum_s[:], func=AF.Copy, scale=1.0 / 256)
    var = sb.tile([1, PQ], F32, tag="ln_var", name="ln_var")
    nc.vector.tensor_scalar(out=var[:], in0=psum_q[:], scalar1=1.0 / 256, scalar2=None, op0=OP.mult)
    m2 = sb.tile([1, PQ], F32, tag="ln_m2", name="ln_m2")
    nc.vector.tensor_tensor(out=m2[:], in0=mean[:], in1=mean[:], op=OP.mult)
    nc.vector.tensor_tensor(out=var[:], in0=var[:], in1=m2[:], op=OP.subtract)
    rv = sb.tile([1, PQ], F32, tag="ln_rv", name="ln_rv")
    nc.vector.tensor_scalar(out=var[:], in0=var[:], scalar1=1e-5, scalar2=None, op0=OP.add)
    nc.vector.reciprocal(rv[:], var[:])
    rstd = sb.tile([1, PQ], F32, tag="ln_rstd", name="ln_rstd")
    nc.scalar.activation(out=rstd[:], in_=rv[:], func=AF.Sqrt)
    # broadcast mean & rstd to 128 partitions via K=1 matmuls
    psum_mb = ps.tile([128, PQ], F32, tag="ln_mb", name="ln_mb")
    psum_rb = ps.tile([128, PQ], F32, tag="ln_rb", name="ln_rb")
    for (o, n) in NSPLIT:
        nc.tensor.matmul(psum_mb[:, o:o + n], onesrow[:], mean[:, o:o + n], start=True, stop=True)
    for (o, n) in NSPLIT:
        nc.tensor.matmul(psum_rb[:, o:o + n], onesrow[:], rstd[:, o:o + n], start=True, stop=True)
    for i in (0, 1):
        t = sb.tile([128, PQ], F32, tag="ln_t", name="ln_t")
        nc.vector.tensor_tensor(out=t[:], in0=x_tiles[i][:], in1=psum_mb[:], op=OP.subtract)
        nc.vector.tensor_tensor(out=t[:], in0=t[:], in1=psum_rb[:], op=OP.mult)
        nc.scalar.activation(out=out32[i][:], in_=t[:], func=AF.Identity,
                             scale=g_ap[:, i:i + 1], bias=be_ap[:, i:i + 1])
        if outbf is not None:
            nc.vector.tensor_copy(out=outbf[i][:], in_=out32[i][:])


def _emit(nc, tc, D, out_d, dbg):
    from contextlib import ExitStack
    _es = ExitStack()
    csts_pool = _es.enter_context(tc.tile_pool(name="consts", bufs=1))
    wpool = _es.enter_context(tc.tile_pool(name="weights", bufs=1))
    bigpool = _es.enter_context(tc.tile_pool(name="big", bufs=1))

    # ---- constants ----
    csts = {}
    iden = csts_pool.tile([128, 128], BF)
    make_identity(nc, iden)
    onesf = csts_pool.tile([128, 1], F32)
    nc.gpsimd.memset(onesf[:], 1.0)
    onesb = csts_pool.tile([128, 1], BF)
    nc.gpsimd.memset(onesb[:], 1.0)
    ind8b = csts_pool.tile([8, 128], BF)  # bf16 head-broadcast indicator
    small = {}
    for n, r, c in CPACK_SPEC:
        t = csts_pool.tile([r, c], F32, tag="c_" + n, name="c_")
        o = CPACK_OFF[n]
        nc.sync.dma_start(out=t[:], in_=D["cpack"][0:r, o:o + c])
        small[n] = t
    csts["onesf"] = onesf
    csts["onesrow"] = small["onesrow"]
    nc.vector.tensor_copy(out=ind8b[:], in_=small["ind_awbc"][:])

    # ---- weights to SBUF (bf16) from packed tensor ----
    W = {}
    for n, s, cols in WPACK_SPEC:
        if n in ("w1T", "w2T"):
            continue  # loaded in the FFN section
        tiles = []
        o = WPACK_OFF[n]
        for k in range(s):
            t = wpool.tile([128, cols], BF, tag=f"w_{n}{k}", name=f"w_{n}{k}")
            nc.sync.dma_start(out=t[:], in_=D["wpack"][:, o + k * cols:o + (k + 1) * cols])
            tiles.append(t)
        W[n] = tiles

    from contextlib import ExitStack as _ES2
    _es2 = _ES2()

    # ======== value projection ========
    vpool = _es2.enter_context(tc.tile_pool(name="vpool", bufs=1))
    vtab = vpool.tile([128, PK], F32, tag="vtab", name="vtab")  # packed bf16-pairs as f32
    vtab_bf = vtab[:].bitcast(BF)                    # [128, 2*PK]
    with tc.tile_pool(name="vkey", bufs=3) as kp, \
         tc.tile_pool(name="vpsum", bufs=2, space="PSUM") as vps:
        off = 0
        for nlen in NCH:
            kb8 = kp.tile([128, 2, 512], F8, tag="keyb8", name="keyb8")
            for k in (0, 1):
                nc.gpsimd.dma_start(out=kb8[:, k, :nlen], in_=D["keyt"][128 * k:128 * (k + 1), off:off + nlen])
            kb = kp.tile([128, 2, 512], BF, tag="keyb", name="keyb")
            nc.scalar.activation(out=kb[:, :, :nlen], in_=kb8[:, :, :nlen], func=AF.Copy)
            for m in (0, 1):  # even-set / odd-set
                pv = vps.tile([128, 512], F32, tag=f"vps{m}", name=f"vps{m}")
                for k in (0, 1):
                    nc.tensor.matmul(pv[:, :nlen], W["wvT"][k][:, 128 * m:128 * (m + 1)],
                                     kb[:, k, :nlen], start=(k == 0), stop=(k == 1))
                ov = vtab_bf[:, 2 * off + m: 2 * (off + nlen): 2]
                nc.scalar.activation(out=ov, in_=pv[:, :nlen], func=AF.Identity,
                                     bias=small["bv"][:, m:m + 1])
            off += nlen


    # ======== self-attention ========
    x1 = [bigpool.tile([128, PQ], F32, tag=f"xr_{i}", name=f"xr_{i}") for i in (0, 1)]
    with tc.tile_pool(name="qkvp", bufs=1) as qp, \
         tc.tile_pool(name="attn_sb", bufs=2) as asb, \
         tc.tile_pool(name="attn_big", bufs=1) as abig, \
         tc.tile_pool(name="attn_ps", bufs=2, space="PSUM") as aps, \
         tc.tile_pool(name="attn_ps1", bufs=1, space="PSUM") as aps1:
        xq32 = []
        xqb = []
        for i in (0, 1):
            tb = abig.tile([128, PQ], BF, tag=f"xqb_{i}", name=f"xqb_{i}")
            nc.sync.dma_start(out=tb[:], in_=D["xq"][128 * i:128 * (i + 1), :])
            xqb.append(tb)
            t = abig.tile([128, PQ], F32, tag=f"xq32_{i}", name=f"xq32_{i}")
            nc.scalar.activation(out=t[:], in_=tb[:], func=AF.Copy)
            xq32.append(t)
        qkvb = []
        for m in range(6):
            pq = aps.tile([128, PQ], F32, tag="psA", name="qkv_ps")
            for (o, n) in NSPLIT:
                for k in (0, 1):
                    nc.tensor.matmul(pq[:, o:o + n], W["winT"][k][:, 128 * m:128 * (m + 1)],
                                     xqb[k][:, o:o + n], start=(k == 0), stop=(k == 1))
            t = qp.tile([128, PQ], BF, tag=f"qkv_{m}", name=f"qkv_{m}")
            nc.scalar.activation(out=t[:], in_=pq[:], func=AF.Identity,
                                 bias=small["b_in"][:, m:m + 1])
            qkvb.append(t)

        SCH = [(0, 128), (128, 128), (256, 128), (384, 128), (512, 128),
               (640, 128), (768, 128), (896, 4)]
        rsum = abig.tile([8, PQ], F32, tag="rsum", name="rsum")
        attn_raw = [abig.tile([128, PQ], BF, tag=f"attnraw{i}", name=f"attnraw{i}") for i in (0, 1)]
        for h in range(NH):
            ti, ro = h // 4, (h % 4) * 32
            q_h = asb.tile([32, PQ], BF, tag="q_h", name="q_h")
            k_h = asb.tile([32, PQ], BF, tag="k_h", name="k_h")
            v_h = asb.tile([32, 1024], BF, tag="v_h", name="v_h")
            nc.sync.dma_start(out=q_h[:], in_=qkvb[0 + ti][ro:ro + 32, :])
            nc.sync.dma_start(out=k_h[:], in_=qkvb[2 + ti][ro:ro + 32, :])
            nc.sync.dma_start(out=v_h[:, :PQ], in_=qkvb[4 + ti][ro:ro + 32, :])
            nc.gpsimd.memset(v_h[:, PQ:], 0.0)
            expS = []
            for s, (so, sn) in enumerate(SCH):
                pS = aps.tile([128, PQ], F32, tag="psA", name="ps_S")
                for (o, n) in NSPLIT:
                    nc.tensor.matmul(pS[:sn, o:o + n], k_h[:, so:so + sn], q_h[:, o:o + n],
                                     start=True, stop=True)
                eS = asb.tile([128, PQ], BF, tag=f"expS{s}", name=f"expS{s}", bufs=2)
                nc.scalar.activation(out=eS[:sn, :], in_=pS[:sn, :], func=AF.Exp,
                                     scale=float(1.0 / np.sqrt(DH)))
                expS.append(eS)
            # sum over keys: bf16 tree + ones matmuls
            b1_ = asb.tile([128, PQ], BF, tag="sum_b1", name="sum_b1")
            b2_ = asb.tile([128, PQ], BF, tag="sum_b2", name="sum_b2")
            b3_ = asb.tile([128, PQ], BF, tag="sum_b3", name="sum_b3")
            nc.vector.tensor_tensor(out=b1_[:], in0=expS[0][:], in1=expS[1][:], op=OP.add)
            nc.vector.tensor_tensor(out=b2_[:], in0=expS[2][:], in1=expS[3][:], op=OP.add)
            nc.vector.tensor_tensor(out=b3_[:], in0=expS[4][:], in1=expS[5][:], op=OP.add)
            nc.vector.tensor_tensor(out=b1_[:], in0=b1_[:], in1=b2_[:], op=OP.add)
            nc.vector.tensor_tensor(out=b1_[:], in0=b1_[:], in1=b3_[:], op=OP.add)
            pssum = aps1.tile([1, PQ], F32, tag="ps_sum", name="ps_sum")
            for (o, n) in NSPLIT:
                nc.tensor.matmul(pssum[:, o:o + n], onesb[:], b1_[:, o:o + n], start=True, stop=False)
                nc.tensor.matmul(pssum[:, o:o + n], onesb[:], expS[6][:, o:o + n], start=False, stop=False)
                nc.tensor.matmul(pssum[:, o:o + n], onesb[0:4, :], expS[7][0:4, o:o + n], start=False, stop=True)
            sums1 = asb.tile([1, PQ], F32, tag="sums1", name="sums1")
            nc.scalar.activation(out=sums1[:], in_=pssum[:], func=AF.Copy)
            nc.sync.dma_start(out=rsum[h:h + 1, :], in_=sums1[:])
            # vT via DMA transpose
            vT = abig.tile([128, 8, 32], BF, tag="vT", name="vT")
            for s, (so, sn) in enumerate(SCH):
                nc.sync.dma_start(out=vT[:128, s, :], in_=v_h[:, so:so + 128], transpose=True)
            # attn @ v (unnormalized)
            pO = aps1.tile([32, PQ], F32, tag="ps_O", name="ps_O")
            for (o, n) in NSPLIT:
                for s, (so, sn) in enumerate(SCH):
                    nc.tensor.matmul(pO[:, o:o + n], vT[:sn, s, :], expS[s][:sn, o:o + n],
                                     start=(s == 0), stop=(s == 7))
            ao_h = asb.tile([32, PQ], BF, tag="ao_h", name="ao_h")
            nc.scalar.activation(out=ao_h[:], in_=pO[:], func=AF.Copy)
            nc.sync.dma_start(out=attn_raw[ti][ro:ro + 32, :], in_=ao_h[:])
        # normalize by 1/rowsum, then W_out + residual
        nc.vector.reciprocal(rsum[:], rsum[:])
        rr = rsum
        attn_n = []
        for i in (0, 1):
            pB = aps.tile([128, PQ], F32, tag="psA", name="ps_bc")
            ind = small["ind_bc0"] if i == 0 else small["ind_bc1"]
            for (o, n) in NSPLIT:
                nc.tensor.matmul(pB[:, o:o + n], ind[:], rr[:, o:o + n], start=True, stop=True)
            t = abig.tile([128, PQ], BF, tag=f"attn_n{i}", name=f"attn_n{i}")
            nc.vector.tensor_tensor(out=t[:], in0=attn_raw[i][:], in1=pB[:], op=OP.mult)
            attn_n.append(t)
        for m in (0, 1):
            pW = aps.tile([128, PQ], F32, tag="psA", name="ps_wout")
            for (o, n) in NSPLIT:
                for k in (0, 1):
                    nc.tensor.matmul(pW[:, o:o + n], W["woutT"][k][:, 128 * m:128 * (m + 1)],
                                     attn_n[k][:, o:o + n], start=(k == 0), stop=(k == 1))
            t = asb.tile([128, PQ], F32, tag="wout_t", name="wout_t", bufs=1)
            nc.scalar.activation(out=t[:], in_=pW[:], func=AF.Identity,
                                 bias=small["b_out"][:, m:m + 1])
            nc.vector.tensor_tensor(out=x1[m][:], in0=xq32[m][:], in1=t[:], op=OP.add)

    # pool for tiles first written after attention (reuses attention SBUF)
    latepool = _es2.enter_context(tc.tile_pool(name="late", bufs=1))

    # ======== LN1 ========
    q1f = [bigpool.tile([128, PQ], F32, tag=f"qf{i}", name=f"q1f{i}") for i in (0, 1)]
    q1b = [bigpool.tile([128, PQ], BF, tag=f"qb{i}", name=f"q1b{i}") for i in (0, 1)]
    with tc.tile_pool(name="ln1", bufs=2) as lsb, \
         tc.tile_pool(name="ln1p", bufs=1, space="PSUM") as lps:
        _ln(nc, (lsb, lps, csts), x1, small["g1"], small["be1"], q1f, q1b)

    if "q1" in dbg:
        for i in (0, 1):
            nc.sync.dma_start(out=dbg["q1"][128 * i:128 * (i + 1), :], in_=q1f[i][:])

    # ======== offsets / attention weights / sampling prep ========
    wdup = []   # per-corner [128, 1800] bf16 (q, c2)-dup
    idx16 = []  # per-corner [128, 900] int16
    with tc.tile_pool(name="samp_sb", bufs=1) as sb, \
         tc.tile_pool(name="samp_ps", bufs=2, space="PSUM") as sps:
        def proj(wname, m, bias, n_out_rows=128):
            ps = sps.tile([128, PQ], F32, tag="proj_ps", name="proj_ps")
            for (o, n) in NSPLIT:
                for k in (0, 1):
                    nc.tensor.matmul(ps[:n_out_rows, o:o + n],
                                     W[wname][k][:, 128 * m:128 * m + n_out_rows],
                                     q1b[k][:, o:o + n], start=(k == 0), stop=(k == 1))
            return ps

        # x / y coordinates: W_off projection and the prescaled-refs
        # expansion accumulate into one psum; bias carries b_off - 0.5.
        refs_t = sb.tile([8, PQ], F32, tag="refs_t", name="refs_t")
        nc.sync.dma_start(out=refs_t[:], in_=D["refs"][:])

        def coord(m, ind_name):
            ps = sps.tile([128, PQ], F32, tag="proj_ps", name="proj_ps")
            for (o, n) in NSPLIT:
                for k in (0, 1):
                    nc.tensor.matmul(ps[:, o:o + n],
                                     W["woffT"][k][:, 128 * m:128 * (m + 1)],
                                     q1b[k][:, o:o + n], start=(k == 0), stop=False)
                nc.tensor.matmul(ps[:, o:o + n], small[ind_name][:], refs_t[:, o:o + n],
                                 start=False, stop=True)
            xv = sb.tile([128, PQ], F32, tag=f"coord_{m}", name=f"coord_{m}")
            nc.scalar.activation(out=xv[:], in_=ps[:], func=AF.Identity,
                                 bias=small["boff"][:, m:m + 1])
            return xv

        x = coord(0, "ind_refx")
        y = coord(1, "ind_refy")

        def split_floor(v, cm1, pfx):
            rnd = sb.tile([128, PQ], F32, tag="sf_rnd", name=f"{pfx}_rnd")
            nc.vector.tensor_scalar(out=rnd[:], in0=v[:], scalar1=8388608.0,
                                    scalar2=8388608.0, op0=OP.add, op1=OP.subtract)
            g_ = sb.tile([128, PQ], F32, tag="sf_g", name=f"{pfx}_g")
            nc.vector.tensor_tensor(out=g_[:], in0=rnd[:], in1=v[:], op=OP.is_gt)
            i0 = sb.tile([128, PQ], F32, tag="sf_i0", name=f"{pfx}_i0")
            nc.vector.tensor_tensor(out=i0[:], in0=rnd[:], in1=g_[:], op=OP.subtract)
            fr = sb.tile([128, PQ], F32, tag=f"{pfx}_fr", name=f"{pfx}_fr")
            nc.vector.tensor_tensor(out=fr[:], in0=v[:], in1=i0[:], op=OP.subtract)
            i0c = sb.tile([128, PQ], F32, tag=f"{pfx}_i0c", name=f"{pfx}_i0c")
            nc.vector.tensor_scalar(out=i0c[:], in0=i0[:], scalar1=0.0, scalar2=small[cm1][:],
                                    op0=OP.max, op1=OP.min)
            v0 = sb.tile([128, PQ], F32, tag=f"{pfx}_v0", name=f"{pfx}_v0")
            nc.vector.tensor_tensor(out=v0[:], in0=i0[:], in1=i0c[:], op=OP.is_equal)
            i1 = sb.tile([128, PQ], F32, tag="sf_i1", name=f"{pfx}_i1")
            nc.vector.tensor_scalar(out=i1[:], in0=i0[:], scalar1=1.0, scalar2=None, op0=OP.add)
            i1c = sb.tile([128, PQ], F32, tag=f"{pfx}_i1c", name=f"{pfx}_i1c")
            nc.vector.tensor_scalar(out=i1c[:], in0=i1[:], scalar1=0.0, scalar2=small[cm1][:],
                                    op0=OP.max, op1=OP.min)
            v1 = sb.tile([128, PQ], F32, tag=f"{pfx}_v1", name=f"{pfx}_v1")
            nc.vector.tensor_tensor(out=v1[:], in0=i1[:], in1=i1c[:], op=OP.is_equal)
            return fr, i0c, v0, i1c, v1

        fx, x0c, vx0, x1c, vx1 = split_floor(x, "cWm1", "x")
        fy, y0c, vy0, y1c, vy1 = split_floor(y, "cHm1", "y")

        # indices
        y0W = sb.tile([128, PQ], F32, tag="coord_0", name="y0W")
        nc.vector.tensor_scalar(out=y0W[:], in0=y0c[:], scalar1=small["cW"][:],
                                scalar2=small["cStart"][:], op0=OP.mult, op1=OP.add)
        y1W = sb.tile([128, PQ], F32, tag="coord_1", name="y1W")
        nc.vector.tensor_scalar(out=y1W[:], in0=y1c[:], scalar1=small["cW"][:],
                                scalar2=small["cStart"][:], op0=OP.mult, op1=OP.add)
        for ci, (yw, xc) in enumerate([(y0W, x0c), (y0W, x1c), (y1W, x0c), (y1W, x1c)]):
            idf = sb.tile([128, PQ], F32, tag="expA", name="idf")
            nc.vector.tensor_tensor(out=idf[:], in0=yw[:], in1=xc[:], op=OP.add)
            ii = latepool.tile([128, PQ], I16, tag=f"idx{ci}", name=f"idx{ci}")
            nc.vector.tensor_copy(out=ii[:], in_=idf[:])
            idx16.append(ii)

        # aw softmax
        psA = proj("wawT", 0, None)
        expA = sb.tile([128, PQ], F32, tag="expA", name="expA")
        nc.scalar.activation(out=expA[:], in_=psA[:], func=AF.Exp, bias=small["baw"][:])
        psGS = sps.tile([8, PQ], F32, tag="ps_gs", name="ps_gs", bufs=1)
        for (o, n) in NSPLIT:
            nc.tensor.matmul(psGS[:, o:o + n], small["ind_awsum"][:], expA[:, o:o + n],
                             start=True, stop=True)
        rGS = sb.tile([8, PQ], F32, tag="rGS", name="rGS")
        nc.vector.reciprocal(rGS[:], psGS[:])
        psGB = sps.tile([128, PQ], F32, tag="ps_gb", name="ps_gb", bufs=1)
        for (o, n) in NSPLIT:
            nc.tensor.matmul(psGB[:, o:o + n], small["ind_awbc"][:], rGS[:, o:o + n],
                             start=True, stop=True)
        awn = sb.tile([128, PQ], F32, tag="awn", name="awn")
        nc.vector.tensor_tensor(out=awn[:], in0=expA[:], in1=psGB[:], op=OP.mult)


        # weights, computed in place into the validity tiles
        wx1t = vx1
        nc.vector.tensor_tensor(out=wx1t[:], in0=fx[:], in1=vx1[:], op=OP.mult)
        omfx = sb.tile([128, PQ], F32, tag="omf", name="omfx")
        nc.scalar.activation(out=omfx[:], in_=fx[:], func=AF.Identity, scale=-1.0, bias=1.0)
        wx0t = vx0
        nc.vector.tensor_tensor(out=wx0t[:], in0=omfx[:], in1=vx0[:], op=OP.mult)
        wy1t = vy1
        nc.vector.tensor_tensor(out=wy1t[:], in0=fy[:], in1=vy1[:], op=OP.mult)
        nc.vector.tensor_tensor(out=wy1t[:], in0=wy1t[:], in1=awn[:], op=OP.mult)
        omfy = sb.tile([128, PQ], F32, tag="omf", name="omfy")
        nc.scalar.activation(out=omfy[:], in_=fy[:], func=AF.Identity, scale=-1.0, bias=1.0)
        wy0t = vy0
        nc.vector.tensor_tensor(out=wy0t[:], in0=omfy[:], in1=vy0[:], op=OP.mult)
        nc.vector.tensor_tensor(out=wy0t[:], in0=wy0t[:], in1=awn[:], op=OP.mult)

        corners = [(wx0t, wy0t), (wx1t, wy0t), (wx0t, wy1t), (wx1t, wy1t)]
        for ci, (wx, wy) in enumerate(corners):
            wd = latepool.tile([128, PQ], BF, tag=f"wdup{ci}", name=f"wdup{ci}")
            nc.vector.tensor_tensor(out=wd[:], in0=wx[:], in1=wy[:], op=OP.mult)
            wdup.append(wd)


        if "aw" in dbg:
            nc.sync.dma_start(out=dbg["aw"][:], in_=awn[:])
        if "xcoord" in dbg:
            nc.sync.dma_start(out=dbg["xcoord"][:], in_=x[:])


    # ======== gather + combine ========
    samp = latepool.tile([128, 2 * PQ], F32, tag="samp", name="samp")  # (q, c2) f32
    with tc.tile_pool(name="gat", bufs=1) as gp, \
         tc.tile_pool(name="gat2", bufs=1) as gp2, \
         tc.tile_pool(name="gat_ps", bufs=2, space="PSUM") as gps:
        HQ = QCH * 8  # cols per lp-half
        for qc in range(NQC):
            q0 = qc * QCH
            S_t = gp2.tile([128, QCH * 16 * 2], BF, tag="S_acc", name="S_acc")
            T_t = gp2.tile([128, QCH * 16 * 2], BF, tag="T_tmp", name="T_tmp")
            for ci in range(4):
                G = gp.tile([128, QCH * 16], F32, tag="G", name="G", bufs=2)
                nc.gpsimd.ap_gather(out_ap=G[:], in_ap=vtab[:], idxs_ap=idx16[ci][:, q0:q0 + QCH],
                                    channels=128, num_elems=PK, d=1, num_idxs=16 * QCH)
                # merge the 16 (l,p) weight rows of each head into one
                # partition (128 dma lines), then replicate across each
                # head's 16 channel-partitions with a K=8 PE matmul.
                M = gp.tile([8, QCH * 16], BF, tag="wmg", name="wmg", bufs=2)
                eng = nc.sync if ci % 2 == 0 else nc.scalar
                eng.dma_start(out=M[:], in_=wdup[ci][:, q0:q0 + QCH])
                Gv = G[:].bitcast(BF).rearrange("p (q lp c) -> p lp q c", q=QCH, lp=16, c=2)
                dst = S_t if ci == 0 else T_t
                dv = dst[:].rearrange("p (lp q c) -> p lp q c", q=QCH, lp=16, c=2)
                for s in (0, 1):
                    P_ = gps.tile([128, HQ], F32, tag="wrep", name="wrep")
                    for o in range(0, HQ, 512):
                        n = min(512, HQ - o)
                        nc.tensor.matmul(P_[:, o:o + n], ind8b[:],
                                         M[:, s * HQ + o:s * HQ + o + n],
                                         start=True, stop=True)
                    Pv = P_[:].rearrange("p (lp q) -> p lp q", lp=8)
                    for c_ in (0, 1):
                        nc.vector.tensor_tensor(out=dv[:, s * 8:(s + 1) * 8, :, c_],
                                                in0=Gv[:, s * 8:(s + 1) * 8, :, c_],
                                                in1=Pv, op=OP.mult)
                if ci > 0:
                    nc.vector.tensor_tensor(out=S_t[:], in0=S_t[:], in1=T_t[:], op=OP.add)
            # lp-tree 16 -> 1, in place on S_t (no extra tiles)
            B_ = QCH * 2
            for half in (8, 4, 2):
                nc.vector.tensor_tensor(out=S_t[:, 0:half * B_],
                                        in0=S_t[:, 0:half * B_],
                                        in1=S_t[:, half * B_:2 * half * B_], op=OP.add)
            nc.vector.tensor_tensor(out=samp[:, 2 * q0:2 * (q0 + QCH)],
                                    in0=S_t[:, 0:B_], in1=S_t[:, B_:2 * B_], op=OP.add)

    if "samp" in dbg:
        nc.sync.dma_start(out=dbg["samp"][:], in_=samp[:])

    # ======== W_o + residual + LN2 ========
    x2 = [bigpool.tile([128, PQ], F32, tag=f"xr_{i}", name=f"x2_{i}") for i in (0, 1)]
    with tc.tile_pool(name="wo_sb", bufs=2) as osb, \
         tc.tile_pool(name="wo_ps", bufs=2, space="PSUM") as ops_:
        sampb = osb.tile([128, 2 * PQ], BF, tag="sampb", name="sampb")
        nc.scalar.activation(out=sampb[:], in_=samp[:], func=AF.Copy)
        sv = sampb[:].rearrange("p (q c) -> p q c", c=2)
        for m in (0, 1):
            pW = ops_.tile([128, PQ], F32, tag="ps_wo", name="ps_wo")
            for (o, n) in NSPLIT:
                nc.tensor.matmul(pW[:, o:o + n], W["woTe"][0][:, 128 * m:128 * (m + 1)],
                                 sv[:, o:o + n, 0], start=True, stop=False)
                nc.tensor.matmul(pW[:, o:o + n], W["woTo"][0][:, 128 * m:128 * (m + 1)],
                                 sv[:, o:o + n, 1], start=False, stop=True)
            t = osb.tile([128, PQ], F32, tag="wo_t", name="wo_t")
            nc.scalar.activation(out=t[:], in_=pW[:], func=AF.Identity,
                                 bias=small["b_o"][:, m:m + 1])
            nc.vector.tensor_tensor(out=x2[m][:], in0=q1f[m][:], in1=t[:], op=OP.add)

    q2f = [bigpool.tile([128, PQ], F32, tag=f"qf{i}", name=f"q2f{i}") for i in (0, 1)]
    q2b = [bigpool.tile([128, PQ], BF, tag=f"qb{i}", name=f"q2b{i}") for i in (0, 1)]
    with tc.tile_pool(name="ln2", bufs=2) as lsb, \
         tc.tile_pool(name="ln2p", bufs=1, space="PSUM") as lps:
        _ln(nc, (lsb, lps, csts), x2, small["g2"], small["be2"], q2f, q2b)

    # ======== FFN + LN3 ========
    x3 = [bigpool.tile([128, PQ], F32, tag=f"xr_{i}", name=f"x3_{i}") for i in (0, 1)]
    with tc.tile_pool(name="ffw", bufs=1) as fw, \
         tc.tile_pool(name="ff_sb", bufs=1) as fsb, \
         tc.tile_pool(name="ff_ps", bufs=2, space="PSUM") as fps:
        w1t = []
        o1 = WPACK_OFF["w1T"]
        for k in (0, 1):
            t = fw.tile([128, FF], BF, tag=f"w1T{k}", name=f"w1T{k}")
            nc.sync.dma_start(out=t[:], in_=D["wpack"][:, o1 + k * FF:o1 + (k + 1) * FF])
            w1t.append(t)
        w2t = []
        o2 = WPACK_OFF["w2T"]
        for k in range(16):
            t = fw.tile([128, C], BF, tag=f"w2T{k}", name=f"w2T{k}")
            nc.sync.dma_start(out=t[:], in_=D["wpack"][:, o2 + k * C:o2 + (k + 1) * C])
            w2t.append(t)
        ffb = []
        for m in range(16):
            pF = fps.tile([128, PQ], F32, tag="ps_ff1", name="ps_ff1")
            for (o, n) in NSPLIT:
                for k in (0, 1):
                    nc.tensor.matmul(pF[:, o:o + n], w1t[k][:, 128 * m:128 * (m + 1)],
                                     q2b[k][:, o:o + n], start=(k == 0), stop=(k == 1))
            t = fsb.tile([128, PQ], BF, tag=f"ff_{m}", name=f"ff_{m}")
            nc.scalar.activation(out=t[:], in_=pF[:], func=AF.Relu,
                                 bias=small["b1"][:, m:m + 1])
            ffb.append(t)
        for m in (0, 1):
            pF2 = fps.tile([128, PQ], F32, tag="ps_ff2", name="ps_ff2")
            for (o, n) in NSPLIT:
                for k in range(16):
                    nc.tensor.matmul(pF2[:, o:o + n], w2t[k][:, 128 * m:128 * (m + 1)],
                                     ffb[k][:, o:o + n], start=(k == 0), stop=(k == 15))
            t = fsb.tile([128, PQ], F32, tag="ff2_t", name="ff2_t")
            nc.scalar.activation(out=t[:], in_=pF2[:], func=AF.Identity,
                                 bias=small["b2"][:, m:m + 1])
            nc.vector.tensor_tensor(out=x3[m][:], in0=q2f[m][:], in1=t[:], op=OP.add)

    outb = [bigpool.tile([128, PQ], BF, tag=f"qb{i}", name=f"o16_{i}") for i in (0, 1)]
    with tc.tile_pool(name="ln3", bufs=2) as lsb, \
         tc.tile_pool(name="ln3p", bufs=1, space="PSUM") as lps:
        _ln(nc, (lsb, lps, csts), x3, small["g3"], small["be3"], outb, None)
    for i in (0, 1):
        nc.sync.dma_start(out=out_d[128 * i:128 * (i + 1), :], in_=outb[i][:])

    _es2.close()
    _es.close()


# ======================================================================
# Self-contained entry point: kernel(**inputs) -> np.ndarray [B, C, PQ]
# Sharding: data-parallel over batch — one sample per NeuronCore (8 cores).
# ======================================================================
_CACHED = {}


def _get_nc():
    if "nc" not in _CACHED:
        _CACHED["nc"] = build()
    return _CACHED["nc"]


def kernel(**inputs) -> np.ndarray:
    from concourse.bass_utils import run_bass_kernel_spmd
    nc = _get_nc()
    in_maps = host_prep(inputs)
    res = run_bass_kernel_spmd(nc, in_maps, core_ids=list(range(B)))
    out = np.stack([res.results[b]["out"] for b in range(B)]).astype(np.float32)
    return out

